# revision 1
# baseline (speedup 1.0000x reference)
"""Llama GQA attention (b=2, s=2048, h=4096, 32 Q heads / 8 KV heads, rope)
as a Bass/Tile kernel for 8 Trainium2 NeuronCores.

Sharding: data-parallel over batch (2) x tensor-parallel over heads (4).
Core c = (b, r), b = c // 4, r = c % 4 handles batch b with Q heads
[8r, 8r+8) and KV heads [2r, 2r+2).  Wq/Wk/Wv column-sharded, Wo
row-sharded; per-core output is a partial sum over the TP group which the
host reduces (fp32 adds).

On-core dataflow (all activations feature-major, i.e. transposed):
  XT [H, T]  --(Wq/Wk stationary, XT moving)-->  QT/KT [heads*128, T] +RoPE
  XT tiles stationary, Wv moving             ->  V  [T, 256] token-major
  S^T[k,q] = KT-tile.T @ QT  (PE), causal masking via additive mask views,
  max-free softmax: exp on ACT (scale=1/sqrt(128) folded in), row sums via
  ones-vector matmuls, normalization applied at O^T eviction through a
  PE outer-product broadcast of 1/sum.
  O^T tiles (== heads of OT) feed Wo projection producing OUT^T [H, T]
  which the host transposes / reduces.
"""

import math
import sys

import numpy as np

for _p in ("/opt/trn_rl_repo",):
    if _p not in sys.path:
        sys.path.insert(0, _p)

import ml_dtypes  # noqa: E402

import concourse.bass as bass  # noqa: E402
import concourse.mybir as mybir  # noqa: E402
import concourse.tile as tile  # noqa: E402
from concourse.alu_op_type import AluOpType  # noqa: E402

F32 = mybir.dt.float32
BF16 = mybir.dt.bfloat16
AF = mybir.ActivationFunctionType

# full problem constants
B, S, H = 2, 2048, 4096
NUM_HEADS, NUM_KV_HEADS, D = 32, 8, 128
ROPE_THETA = 10000.0
TP, DP = 4, 2
MASK_VAL = -30000.0


def build_nc(T=S, HID=H, NQL=NUM_HEADS // TP, NKVL=NUM_KV_HEADS // TP,
             HOUT=H, TQ=512):
    """One-core SPMD program.  T tokens, HID hidden, NQL local Q heads,
    NKVL local KV heads, HOUT output features, TQ q-tile width."""
    assert T % TQ == 0 and TQ % 128 == 0 and HID % 128 == 0
    GRP = NQL // NKVL            # q heads per kv head
    QC = NQL * D                 # local q columns
    KC = NKVL * D                # local kv columns
    KX = HID // 128              # contraction chunks for projections
    NTOK = T // TQ               # token tiles of width TQ
    NT128 = T // 128             # token tiles of width 128
    NKT = TQ // 128              # 128-wide k tiles per q tile
    MW = TQ + (TQ - 128)         # additive causal mask width

    nc = bass.Bass()
    xt = nc.dram_tensor("xt", [HID, T], BF16, kind="ExternalInput")
    wq = nc.dram_tensor("wq", [HID, QC], BF16, kind="ExternalInput")
    wk = nc.dram_tensor("wk", [HID, KC], BF16, kind="ExternalInput")
    wv = nc.dram_tensor("wv", [HID, KC], BF16, kind="ExternalInput")
    wo = nc.dram_tensor("wo", [QC, HOUT], BF16, kind="ExternalInput")
    cosb = nc.dram_tensor("cosb", [128, T], F32, kind="ExternalInput")
    sinb = nc.dram_tensor("sinb", [128, T], F32, kind="ExternalInput")  # sign-folded
    maskb = nc.dram_tensor("maskb", [128, MW], F32, kind="ExternalInput")
    outp = nc.dram_tensor("outp", [HOUT, T], F32, kind="ExternalOutput")

    with tile.TileContext(nc) as tc:
        with (
            tc.tile_pool(name="resident", bufs=1) as res,
            tc.tile_pool(name="const", bufs=1) as const,
        ):
            # resident SBUF arrays
            qt = [res.tile([128, T], BF16, tag=f"qt{h}", name=f"qt{h}") for h in range(NQL)]
            kt = [res.tile([128, T], BF16, tag=f"kt{h}", name=f"kt{h}") for h in range(NKVL)]
            vt = [res.tile([128, KC], BF16, tag=f"v{t}", name=f"v{t}") for t in range(NT128)]
            ot = [res.tile([128, T], BF16, tag=f"ot{h}", name=f"ot{h}") for h in range(NQL)]
            cos_sb = res.tile([128, T], F32, tag="cos")
            sin_sb = res.tile([128, T], F32, tag="sin")
            mask_sb = res.tile([128, MW], F32, tag="mask")
            nc.sync.dma_start(out=cos_sb[:], in_=cosb[:])
            nc.sync.dma_start(out=sin_sb[:], in_=sinb[:])
            nc.sync.dma_start(out=mask_sb[:], in_=maskb[:])
            ones_col = const.tile([128, 1], BF16)
            ones_row = const.tile([1, 128], BF16)
            nc.vector.memset(ones_col[:], 1.0)
            nc.vector.memset(ones_row[:], 1.0)

            # ---------------- phase 1: projections ----------------
            with (
                tc.tile_pool(name="xk", bufs=3) as xpool,
                tc.tile_pool(name="wt", bufs=3) as wpool,
                tc.tile_pool(name="rope_tmp", bufs=3) as rpool,
                tc.tile_pool(name="pj_psum", bufs=1, space="PSUM") as pp,
            ):
                def rope_evict(ps, dst_ap, tok0):
                    """dst = ps*cos + rot_half(ps)*sin  (sin sign-folded)."""
                    cw = cos_sb[:, tok0:tok0 + TQ]
                    sw = sin_sb[:, tok0:tok0 + TQ]
                    r = rpool.tile([128, TQ], F32, tag="rot")
                    nc.scalar.copy(r[0:64, :], ps[64:128, :])
                    nc.scalar.copy(r[64:128, :], ps[0:64, :])
                    t1 = rpool.tile([128, TQ], F32, tag="t1")
                    nc.vector.tensor_tensor(t1[:], ps[:], cw, op=AluOpType.mult)
                    nc.vector.tensor_tensor(r[:], r[:], sw, op=AluOpType.mult)
                    nc.vector.tensor_tensor(dst_ap, t1[:], r[:], op=AluOpType.add)

                # Q and K sweeps: 2 head-columns at a time, out = W.T @ XT
                qk_jobs = [("q", wq, qt, h) for h in range(0, NQL, 2)] + \
                          [("k", wk, kt, h) for h in range(0, NKVL, 2)]
                for _, wsrc, dst, h0 in qk_jobs:
                    nheads = min(2, len(dst) - h0)
                    ps = [[pp.tile([128, TQ], F32, tag=f"pp{i * NTOK + j}", name=f"pj{i}{j}")
                           for j in range(NTOK)] for i in range(nheads)]
                    for k in range(KX):
                        xk = xpool.tile([128, T], BF16, tag="xk")
                        nc.sync.dma_start(out=xk[:], in_=xt[k * 128:(k + 1) * 128, :])
                        wt_sb = wpool.tile([128, nheads * 128], BF16, tag="w")
                        nc.sync.dma_start(
                            out=wt_sb[:],
                            in_=wsrc[k * 128:(k + 1) * 128,
                                     h0 * 128:(h0 + nheads) * 128])
                        for i in range(nheads):
                            for j in range(NTOK):
                                nc.tensor.matmul(
                                    ps[i][j][:],
                                    lhsT=wt_sb[:, i * 128:(i + 1) * 128],
                                    rhs=xk[:, j * TQ:(j + 1) * TQ],
                                    start=(k == 0), stop=(k == KX - 1))
                    for i in range(nheads):
                        for j in range(NTOK):
                            rope_evict(ps[i][j], dst[h0 + i][:, j * TQ:(j + 1) * TQ],
                                       j * TQ)

                # V sweeps: out[t, c] token-major; XT tiles stationary
                VG = min(8, NT128)
                for g0 in range(0, NT128, VG):
                    gn = min(VG, NT128 - g0)
                    psv = [pp.tile([128, KC], F32, tag=f"pp{i}", name=f"pv{i}") for i in range(gn)]
                    for k in range(KX):
                        xk = xpool.tile([128, T], BF16, tag="xk")
                        nc.sync.dma_start(out=xk[:], in_=xt[k * 128:(k + 1) * 128, :])
                        wv_sb = wpool.tile([128, KC], BF16, tag="wv")
                        nc.sync.dma_start(out=wv_sb[:],
                                          in_=wv[k * 128:(k + 1) * 128, :])
                        for i in range(gn):
                            t0 = (g0 + i) * 128
                            nc.tensor.matmul(
                                psv[i][:], lhsT=xk[:, t0:t0 + 128], rhs=wv_sb[:],
                                start=(k == 0), stop=(k == KX - 1))
                    for i in range(gn):
                        nc.vector.tensor_copy(vt[g0 + i][:], psv[i][:])

            # ---------------- phase 2: attention ----------------
            # Per head: pass1 emits all S^T matmuls (k-outer for KT ldweights
            # reuse) + exps; pass2 runs AV+sum accumulation per q-tile with
            # the 1/sum broadcast-normalize of q-tile qi deferred until after
            # AV of qi+1, so the DVE recip chain never stalls the PE stream.
            inv_sqrt_d = 1.0 / math.sqrt(D)
            with (
                tc.tile_pool(name="es", bufs=1) as epool,
                tc.tile_pool(name="at_small", bufs=4) as spool,
                tc.tile_pool(name="ps_s", bufs=3, space="PSUM") as psum_s,
                tc.tile_pool(name="ps_o", bufs=2, space="PSUM") as psum_o,
                tc.tile_pool(name="ps_n", bufs=2, space="PSUM") as psum_n,
            ):
                for h in range(NQL):
                    kvh = h // GRP
                    nks = [(qi + 1) * NKT for qi in range(NTOK)]
                    es = {}
                    # pass1: S^T + exp, k-tile outer so KT ldweights get reuse
                    for ki in range(nks[-1]):
                        for qi in range(NTOK):
                            if ki >= nks[qi]:
                                continue
                            q0 = qi * TQ
                            ps_s = psum_s.tile([128, TQ], F32, tag="s")
                            nc.tensor.matmul(
                                ps_s[:], lhsT=kt[kvh][:, ki * 128:(ki + 1) * 128],
                                rhs=qt[h][:, q0:q0 + TQ], start=True, stop=True)
                            if ki >= nks[qi] - NKT:  # diagonal band: add mask
                                off = ki * 128 - q0
                                mv = mask_sb[:, (TQ - 128) - off:
                                             (TQ - 128) - off + TQ]
                                nc.vector.tensor_tensor(ps_s[:], ps_s[:], mv,
                                                        op=AluOpType.add)
                            e = epool.tile([128, TQ], BF16, tag=f"e{qi}_{ki}",
                                           name=f"e{qi}_{ki}")
                            nc.scalar.activation(e[:], ps_s[:], AF.Exp,
                                                 scale=inv_sqrt_d)
                            es[(qi, ki)] = e

                    # pass2: AV + sums per q-tile; normalize of q-tile qi is
                    # deferred one q-tile so the recip chain is off PE's path
                    pending = []

                    def flush_pending():
                        ps_o_, rcb_, q0_ = pending.pop(0)
                        ps_b = psum_s.tile([128, TQ], F32, tag="s", name="ps_b")
                        nc.tensor.matmul(ps_b[:], lhsT=ones_row[:], rhs=rcb_[:],
                                         start=True, stop=True)
                        bc = spool.tile([128, TQ], F32, tag="bc", name="bc")
                        nc.vector.tensor_copy(bc[:], ps_b[:])
                        nc.vector.tensor_tensor(ot[h][:, q0_:q0_ + TQ],
                                                ps_o_[:], bc[:],
                                                op=AluOpType.mult)

                    for qi in range(NTOK):
                        nk = nks[qi]
                        q0 = qi * TQ
                        ps_o = psum_o.tile([128, TQ], F32, tag="o")
                        ps_sum = psum_n.tile([1, TQ], F32, tag="sum")
                        for ki in range(nk):
                            nc.tensor.matmul(
                                ps_o[:], lhsT=vt[ki][:, kvh * D:(kvh + 1) * D],
                                rhs=es[(qi, ki)][:],
                                start=(ki == 0), stop=(ki == nk - 1))
                            nc.tensor.matmul(
                                ps_sum[:], lhsT=ones_col[:], rhs=es[(qi, ki)][:],
                                start=(ki == 0), stop=(ki == nk - 1))
                        rc = spool.tile([1, TQ], F32, tag="rc")
                        nc.vector.reciprocal(rc[:], ps_sum[:])
                        rcb = spool.tile([1, TQ], BF16, tag="rcb")
                        nc.vector.tensor_copy(rcb[:], rc[:])
                        pending.append((ps_o, rcb, q0))
                        if len(pending) > 1:
                            flush_pending()
                    while pending:
                        flush_pending()

            # ---------------- phase 3: output projection ----------------
            CT = QC // 128  # contraction chunks (== NQL)
            with (
                tc.tile_pool(name="wo_sb", bufs=2) as wopool,
                tc.tile_pool(name="ob", bufs=4) as obpool,
                tc.tile_pool(name="po_psum", bufs=2, space="PSUM") as pop,
            ):
                NG = 4  # n-tiles per weight fetch group
                for ng in range(0, HOUT // 128, NG):
                    gn = min(NG, HOUT // 128 - ng)
                    wos = []
                    for c in range(CT):
                        w = wopool.tile([128, gn * 128], BF16, tag=f"wo{c}", name=f"wosb{c}")
                        nc.sync.dma_start(
                            out=w[:], in_=wo[c * 128:(c + 1) * 128,
                                            ng * 128:(ng + gn) * 128])
                        wos.append(w)
                    for i in range(gn):
                        ni = ng + i
                        ps = [pop.tile([128, TQ], F32, tag=f"po{j}", name=f"po{j}")
                              for j in range(NTOK)]
                        for c in range(CT):
                            for j in range(NTOK):
                                nc.tensor.matmul(
                                    ps[j][:], lhsT=wos[c][:, i * 128:(i + 1) * 128],
                                    rhs=ot[c][:, j * TQ:(j + 1) * TQ],
                                    start=(c == 0), stop=(c == CT - 1))
                        for j in range(NTOK):
                            ob = obpool.tile([128, TQ], F32, tag="ob")
                            nc.scalar.copy(ob[:], ps[j][:])
                            nc.sync.dma_start(
                                out=outp[ni * 128:(ni + 1) * 128,
                                         j * TQ:(j + 1) * TQ],
                                in_=ob[:])
    legalize_wait_counts(nc)
    return nc


def legalize_wait_counts(nc):
    """walrus DIRECT2D descriptors accept a single sync-wait; Tile can emit
    more (data wait + queue-head wait).  Hoist excess waits onto
    EventSemaphore instructions inserted just before, on the same engine."""
    n_new = 0
    for f in nc.m.functions:
        for blk in f.blocks:
            idx = 0
            insts = blk.instructions
            while idx < len(insts):
                inst = insts[idx]
                si = getattr(inst, "sync_info", None)
                cap = 2 if isinstance(inst, mybir.InstEventSemaphore) else 1
                waits = list(si.on_wait) if si is not None and si.on_wait else []
                if len(waits) > cap:
                    keep, extra = waits[-cap:], waits[:-cap]
                    si.on_wait = keep
                    for i in range(0, len(extra), 2):
                        ev = mybir.InstEventSemaphore(
                            name=f"waitsplit_{n_new}", ins=[], outs=[])
                        n_new += 1
                        ev.engine = inst.engine
                        ev.sync_info = mybir.SyncInfo(
                            on_wait=extra[i:i + 2], on_update=[])
                        nc.register_instruction(ev)
                        insts.insert(idx, ev)
                        idx += 1
                idx += 1
    return n_new


def _host_inputs(hidden_states, position_ids, Wq, Wk, Wv, Wo):
    """Build the 8 per-core input maps."""
    hs = np.asarray(hidden_states, dtype=np.float32)
    pos = np.asarray(position_ids)
    Wq = np.asarray(Wq, dtype=np.float32)
    Wk = np.asarray(Wk, dtype=np.float32)
    Wv = np.asarray(Wv, dtype=np.float32)
    Wo = np.asarray(Wo, dtype=np.float32)
    b, s, h = hs.shape
    qc = h // TP
    kc = (NUM_KV_HEADS * D) // TP
    bf = ml_dtypes.bfloat16

    # rope tables per batch, feature-major, sin sign-folded for rotate_half
    inv_freq = 1.0 / (ROPE_THETA ** (np.arange(0, D, 2, dtype=np.float32) / D))
    maps = []
    TQ = 512
    mw = TQ + (TQ - 128)
    i_idx = np.arange(128)[:, None]
    m_idx = np.arange(mw)[None, :]
    maskb = np.where(m_idx >= i_idx + (TQ - 128), 0.0, MASK_VAL).astype(np.float32)

    for c in range(DP * TP):
        bb, r = c // TP, c % TP
        t = pos[bb].astype(np.float64)  # [s]
        ang = t[None, :] * np.concatenate([inv_freq, inv_freq])[:, None]  # [128, s]
        cosb = np.cos(ang).astype(np.float32)
        sinb = np.sin(ang).astype(np.float32)
        sinb[0:64, :] *= -1.0  # rotate_half sign fold
        maps.append({
            "xt": np.ascontiguousarray(hs[bb].T).astype(bf),
            "wq": np.ascontiguousarray(Wq[:, r * qc:(r + 1) * qc]).astype(bf),
            "wk": np.ascontiguousarray(Wk[:, r * kc:(r + 1) * kc]).astype(bf),
            "wv": np.ascontiguousarray(Wv[:, r * kc:(r + 1) * kc]).astype(bf),
            "wo": np.ascontiguousarray(Wo[r * qc:(r + 1) * qc, :]).astype(bf),
            "cosb": cosb,
            "sinb": sinb,
            "maskb": maskb,
        })
    return maps


_NC_CACHE = {}


def _get_nc():
    if "nc" not in _NC_CACHE:
        _NC_CACHE["nc"] = build_nc()
    return _NC_CACHE["nc"]


def kernel(hidden_states, position_ids, Wq, Wk, Wv, Wo, _results_hook=None):
    from concourse.bass_utils import run_bass_kernel_spmd

    maps = _host_inputs(hidden_states, position_ids, Wq, Wk, Wv, Wo)
    nc = _get_nc()
    res = run_bass_kernel_spmd(nc, maps, list(range(DP * TP)))
    if _results_hook is not None:
        _results_hook(res)
    b, s, h = np.asarray(hidden_states).shape
    out = np.zeros((b, s, h), dtype=np.float32)
    for c in range(DP * TP):
        bb = c // TP
        out[bb] += res.results[c]["outp"].T
    return out


if __name__ == "__main__":
    # smoke: build the full-size program and print instruction counts
    nc = build_nc()
    print("built ok")



# revision 8
# speedup vs baseline: 1.0476x; 1.0476x over previous
"""Llama GQA attention (b=2, s=2048, h=4096, 32 Q heads / 8 KV heads, rope)
as a Bass/Tile kernel for 8 Trainium2 NeuronCores.

Sharding: data-parallel over batch (2) x tensor-parallel over heads (4).
Core c = (b, r), b = c // 4, r = c % 4 handles batch b with Q heads
[8r, 8r+8) and KV heads [2r, 2r+2).  Wq/Wk/Wv column-sharded, Wo
row-sharded; per-core output is a partial sum over the TP group which the
host reduces (fp32 adds).

On-core dataflow (all activations feature-major, i.e. transposed):
  XT [H, T] is loaded ONCE into SBUF (resident) and swept three times
  with weights streaming: K sweep, V sweep (token-major, i-outer so it
  pipelines against K's rope evictions), Q sweeps (one head per sweep,
  alternating PSUM rings).  RoPE is applied on PSUM eviction.

  Attention runs per 512-wide q-tile with a software pipeline over heads:
  at step h the PE emits S^T(h) tiles while the row-sum + AV matmuls of
  head h-1 consume the exp'd tiles, so the scalar engine's exp stream
  (the slow stage) is never on the PE's critical path.  Causal masking
  uses a single [128,128] additive diagonal block; fully-masked 128-col
  sub-blocks of band tiles are skipped via partial-width matmuls/exps.
  Softmax is max-free; 1/rowsum via reciprocal_approx_fast, broadcast
  through a PE outer product.

  The Wo projection is interleaved after q-tiles 1 and 3 (all heads'
  O^T columns for those tiles are complete), which keeps the PE warm and
  amortizes weight loads over two 512-wide rhs tiles.
"""

import math
import sys

import numpy as np

for _p in ("/opt/trn_rl_repo",):
    if _p not in sys.path:
        sys.path.insert(0, _p)

import ml_dtypes  # noqa: E402

import concourse.bass as bass  # noqa: E402
import concourse.mybir as mybir  # noqa: E402
import concourse.tile as tile  # noqa: E402
from concourse.alu_op_type import AluOpType  # noqa: E402

F32 = mybir.dt.float32
BF16 = mybir.dt.bfloat16
AF = mybir.ActivationFunctionType

# full problem constants
B, S, H = 2, 2048, 4096
NUM_HEADS, NUM_KV_HEADS, D = 32, 8, 128
ROPE_THETA = 10000.0
TP, DP = 4, 2
MASK_VAL = -30000.0


def build_nc(T=S, HID=H, NQL=NUM_HEADS // TP, NKVL=NUM_KV_HEADS // TP,
             HOUT=H, TQ=512):
    """One-core SPMD program.  T tokens, HID hidden, NQL local Q heads,
    NKVL local KV heads, HOUT output features, TQ q-tile width."""
    assert T % TQ == 0 and TQ % 128 == 0 and HID % 128 == 0
    GRP = NQL // NKVL            # q heads per kv head
    QC = NQL * D                 # local q columns
    KC = NKVL * D                # local kv columns
    KX = HID // 128              # contraction chunks for projections
    NTOK = T // TQ               # token tiles of width TQ
    NT128 = T // 128             # token tiles of width 128
    NKT = TQ // 128              # 128-wide k tiles per q tile
    inv_sqrt_d = 1.0 / math.sqrt(D)

    nc = bass.Bass()
    xt = nc.dram_tensor("xt", [HID, T], BF16, kind="ExternalInput")
    wq = nc.dram_tensor("wq", [HID, QC], BF16, kind="ExternalInput")
    wk = nc.dram_tensor("wk", [HID, KC], BF16, kind="ExternalInput")
    wv = nc.dram_tensor("wv", [HID, KC], BF16, kind="ExternalInput")
    wo = nc.dram_tensor("wo", [QC, HOUT], BF16, kind="ExternalInput")
    cosb = nc.dram_tensor("cosb", [128, T], BF16, kind="ExternalInput")
    sinb = nc.dram_tensor("sinb", [128, T], BF16, kind="ExternalInput")  # sign-folded
    diagb = nc.dram_tensor("diagb", [128, 128], F32, kind="ExternalInput")
    outp = nc.dram_tensor("outp", [HOUT, T], F32, kind="ExternalOutput")

    with tile.TileContext(nc) as tc:
        with (
            tc.tile_pool(name="resident", bufs=1) as res,
            tc.tile_pool(name="const", bufs=1) as const,
        ):
            # resident SBUF arrays (live across the whole kernel)
            qt = [res.tile([128, T], BF16, tag=f"qt{h}", name=f"qt{h}") for h in range(NQL)]
            kt = [res.tile([128, T], BF16, tag=f"kt{h}", name=f"kt{h}") for h in range(NKVL)]
            vt = [res.tile([128, KC], BF16, tag=f"v{t}", name=f"v{t}") for t in range(NT128)]
            cos_sb = res.tile([128, T], BF16, tag="cos")
            sin_sb = res.tile([128, T], BF16, tag="sin")
            diag_sb = res.tile([128, 128], F32, tag="diag")
            nc.sync.dma_start(out=cos_sb[:], in_=cosb[:])
            nc.sync.dma_start(out=sin_sb[:], in_=sinb[:])
            nc.sync.dma_start(out=diag_sb[:], in_=diagb[:])
            ones_col = const.tile([128, 1], BF16)
            ones_row = const.tile([1, 128], BF16)
            nc.vector.memset(ones_col[:], 1.0)
            nc.vector.memset(ones_row[:], 1.0)

            # ---------------- phase 1: projections ----------------
            with (
                tc.tile_pool(name="xres", bufs=1) as xres,
                tc.tile_pool(name="wt", bufs=1) as wpool,
                tc.tile_pool(name="rope_tmp", bufs=1) as rpool,
            ):
                # resident XT: one HBM read for all sweeps
                xts = [xres.tile([128, T], BF16, tag=f"x{k}", name=f"x{k}")
                       for k in range(KX)]
                for k in range(KX):
                    nc.sync.dma_start(out=xts[k][:], in_=xt[k * 128:(k + 1) * 128, :])

                def rope_evict(ps, dst_ap, tok0):
                    """dst = ps*cos + rot_half(ps)*sin  (sin sign-folded)."""
                    cw = cos_sb[:, tok0:tok0 + TQ]
                    sw = sin_sb[:, tok0:tok0 + TQ]
                    r = rpool.tile([128, TQ], F32, tag="rot", bufs=2, name="rot")
                    nc.scalar.copy(r[0:64, :], ps[64:128, :])
                    nc.scalar.copy(r[64:128, :], ps[0:64, :])
                    t1 = rpool.tile([128, TQ], F32, tag="t1", bufs=2, name="t1")
                    nc.vector.tensor_tensor(t1[:], ps[:], cw, op=AluOpType.mult)
                    nc.vector.tensor_tensor(r[:], r[:], sw, op=AluOpType.mult)
                    nc.vector.tensor_tensor(dst_ap, t1[:], r[:], op=AluOpType.add)

                # V sweep first (no prior-eviction PSUM waits): token-major,
                # k-outer, two groups of 8 banks
                with tc.tile_pool(name="pv", bufs=1, space="PSUM") as pv:
                    for g in range(NT128 // 8):
                        psv = [pv.tile([128, KC], F32, tag=f"pv{i}", name=f"psv{i}")
                               for i in range(8)]
                        for k in range(KX):
                            wv_sb = wpool.tile([128, KC], BF16, tag="wv", bufs=3,
                                               name="wv_sb")
                            nc.sync.dma_start(out=wv_sb[:],
                                              in_=wv[k * 128:(k + 1) * 128, :])
                            for i in range(8):
                                t0 = (g * 8 + i) * 128
                                nc.tensor.matmul(
                                    psv[i][:], lhsT=xts[k][:, t0:t0 + 128],
                                    rhs=wv_sb[:],
                                    start=(k == 0), stop=(k == KX - 1))
                        for i in range(8):
                            nc.scalar.copy(vt[g * 8 + i][:], psv[i][:])

                # K sweep: 2 heads x 4 token tiles = 8 PSUM banks, k-outer
                with tc.tile_pool(name="pk", bufs=1, space="PSUM") as pk:
                    psk = [[pk.tile([128, TQ], F32, tag=f"pk{i}{j}", name=f"pk{i}{j}")
                            for j in range(NTOK)] for i in range(NKVL)]
                    for k in range(KX):
                        wk_sb = wpool.tile([128, KC], BF16, tag="wk", bufs=3, name="wk_sb")
                        nc.sync.dma_start(out=wk_sb[:], in_=wk[k * 128:(k + 1) * 128, :])
                        for i in range(NKVL):
                            for j in range(NTOK):
                                nc.tensor.matmul(
                                    psk[i][j][:],
                                    lhsT=wk_sb[:, i * 128:(i + 1) * 128],
                                    rhs=xts[k][:, j * TQ:(j + 1) * TQ],
                                    start=(k == 0), stop=(k == KX - 1))
                    for i in range(NKVL):
                        for j in range(NTOK):
                            rope_evict(psk[i][j], kt[i][:, j * TQ:(j + 1) * TQ], j * TQ)

                # Q sweeps: one head per sweep, 4 banks, alternating rings
                with tc.tile_pool(name="pq", bufs=1, space="PSUM") as pq:
                    for h in range(NQL):
                        psq = [pq.tile([128, TQ], F32, tag=f"pq{j}", bufs=2,
                                       name=f"psq{j}") for j in range(NTOK)]
                        for k in range(KX):
                            wq_sb = wpool.tile([128, 128], BF16, tag="wq", bufs=3,
                                               name="wq_sb")
                            nc.sync.dma_start(
                                out=wq_sb[:],
                                in_=wq[k * 128:(k + 1) * 128, h * 128:(h + 1) * 128])
                            for j in range(NTOK):
                                nc.tensor.matmul(
                                    psq[j][:], lhsT=wq_sb[:],
                                    rhs=xts[k][:, j * TQ:(j + 1) * TQ],
                                    start=(k == 0), stop=(k == KX - 1))
                        for j in range(NTOK):
                            rope_evict(psq[j], qt[h][:, j * TQ:(j + 1) * TQ], j * TQ)

            # -------- phase 2+3: attention with interleaved Wo --------
            CT = QC // 128  # Wo contraction chunks (== NQL)
            with (
                tc.tile_pool(name="ot_pool", bufs=1) as otpool,
                tc.tile_pool(name="wo_sb", bufs=1) as wopool,
                tc.tile_pool(name="es", bufs=1) as epool,
                tc.tile_pool(name="at_small", bufs=1) as spool,
                tc.tile_pool(name="ob", bufs=1) as obpool,
            ):
                ot = [otpool.tile([128, T], BF16, tag=f"ot{h}", name=f"ot{h}")
                      for h in range(NQL)]
                wos = [wopool.tile([128, HOUT], BF16, tag=f"wo{c}", name=f"wos{c}")
                       for c in range(CT)]
                for c in range(CT):
                    nc.sync.dma_start(out=wos[c][:], in_=wo[c * 128:(c + 1) * 128, :])

                for qi in range(NTOK):
                    nk = (qi + 1) * NKT
                    q0 = qi * TQ
                    with (
                        tc.tile_pool(name="ps_s", bufs=1, space="PSUM") as psum_s,
                        tc.tile_pool(name="ps_o", bufs=1, space="PSUM") as psum_o,
                        tc.tile_pool(name="ps_n", bufs=1, space="PSUM") as psum_n,
                    ):
                        es = {}
                        ctx = {}
                        pending = []

                        def col0(ki):
                            j = ki - qi * NKT
                            return 128 * j if j > 0 else 0

                        def flush_pending():
                            hp, ps_o, ps_n = pending.pop(0)
                            rc = spool.tile([1, TQ], F32, tag="rc", bufs=2,
                                            name="rc")
                            nc.vector.reciprocal(rc[:], ps_n[0:1, :])
                            rcb = spool.tile([1, TQ], BF16, tag="rcb", bufs=2,
                                             name="rcb")
                            nc.scalar.copy(rcb[:], rc[:])
                            ps_b = psum_s.tile([128, TQ], F32, tag="s", bufs=4,
                                               name="ps_b")
                            nc.tensor.matmul(ps_b[:], lhsT=ones_row[:],
                                             rhs=rcb[:], start=True, stop=True)
                            bc = spool.tile([128, TQ], BF16, tag="bc", bufs=2,
                                            name="bc")
                            nc.scalar.copy(bc[:], ps_b[:])
                            nc.vector.tensor_tensor(
                                ot[hp][:, q0:q0 + TQ], ps_o[:], bc[:],
                                op=AluOpType.mult)

                        for hr in range(NQL + 1):
                            h = hr if hr < NQL else None          # producer
                            hc = hr - 1 if hr >= 1 else None      # consumer
                            if hc is not None:
                                ps_o = psum_o.tile([128, TQ], F32, tag="o", bufs=2,
                                                   name="ps_o")
                                ps_n = psum_n.tile([1, TQ], F32, tag="n", bufs=2,
                                                   name="ps_n")
                                ctx[hc] = (ps_o, ps_n)
                            for ki in range(nk):
                                c0 = col0(ki)
                                if h is not None:
                                    kvh = h // GRP
                                    ps_s = psum_s.tile([128, TQ], F32, tag="s",
                                                       bufs=4, name="ps_s")
                                    nc.tensor.matmul(
                                        ps_s[:, c0:TQ],
                                        lhsT=kt[kvh][:, ki * 128:(ki + 1) * 128],
                                        rhs=qt[h][:, q0 + c0:q0 + TQ],
                                        start=True, stop=True)
                                    if ki >= qi * NKT:  # diagonal block mask
                                        nc.vector.tensor_tensor(
                                            ps_s[:, c0:c0 + 128], ps_s[:, c0:c0 + 128],
                                            diag_sb[:], op=AluOpType.add)
                                    e = epool.tile([128, TQ], BF16,
                                                   tag=f"e{h % 2}_{ki}", name="e")
                                    nc.scalar.activation(e[:, c0:TQ], ps_s[:, c0:TQ],
                                                         AF.Exp, scale=inv_sqrt_d)
                                    es[(h % 2, ki)] = e
                                if hc is not None:
                                    kvc = hc // GRP
                                    ps_o, ps_n = ctx[hc]
                                    ec = es[(hc % 2, ki)]
                                    nc.tensor.matmul(
                                        ps_n[0:1, c0:TQ], lhsT=ones_col[:],
                                        rhs=ec[:, c0:TQ],
                                        start=(ki == 0), stop=(ki == nk - 1))
                                    nc.tensor.matmul(
                                        ps_o[:, c0:TQ],
                                        lhsT=vt[ki][:, kvc * D:(kvc + 1) * D],
                                        rhs=ec[:, c0:TQ],
                                        start=(ki == 0), stop=(ki == nk - 1))
                            if hc is not None:
                                pending.append((hc, *ctx.pop(hc)))
                                if len(pending) > 1:
                                    flush_pending()
                        while pending:
                            flush_pending()

                    # Wo for token tiles {qi-1, qi} once both are complete
                    if qi % 2 == 1:
                        jb = qi - 1
                        with tc.tile_pool(name="po", bufs=1, space="PSUM") as pop:
                            for ni in range(HOUT // 128):
                                ps = [pop.tile([128, TQ], F32, tag=f"po{ni % 2}{jj}",
                                               bufs=2, name=f"po{jj}")
                                      for jj in range(2)]
                                for c in range(CT):
                                    for jj in range(2):
                                        nc.tensor.matmul(
                                            ps[jj][:],
                                            lhsT=wos[c][:, ni * 128:(ni + 1) * 128],
                                            rhs=ot[c][:, (jb + jj) * TQ:
                                                      (jb + jj + 1) * TQ],
                                            start=(c == 0), stop=(c == CT - 1))
                                for jj in range(2):
                                    ob = obpool.tile([128, TQ], F32, tag="ob",
                                                     bufs=4, name="ob")
                                    nc.scalar.copy(ob[:], ps[jj][:])
                                    nc.sync.dma_start(
                                        out=outp[ni * 128:(ni + 1) * 128,
                                                 (jb + jj) * TQ:(jb + jj + 1) * TQ],
                                        in_=ob[:])
    legalize_wait_counts(nc)
    return nc


def legalize_wait_counts(nc):
    """walrus DIRECT2D descriptors accept a single sync-wait; Tile can emit
    more (data wait + queue-head wait).  Hoist excess waits onto
    EventSemaphore instructions inserted just before, on the same engine."""
    n_new = 0
    for f in nc.m.functions:
        for blk in f.blocks:
            idx = 0
            insts = blk.instructions
            while idx < len(insts):
                inst = insts[idx]
                si = getattr(inst, "sync_info", None)
                cap = 2 if isinstance(inst, mybir.InstEventSemaphore) else 1
                waits = list(si.on_wait) if si is not None and si.on_wait else []
                if len(waits) > cap:
                    keep, extra = waits[-cap:], waits[:-cap]
                    si.on_wait = keep
                    for i in range(0, len(extra), 2):
                        ev = mybir.InstEventSemaphore(
                            name=f"waitsplit_{n_new}", ins=[], outs=[])
                        n_new += 1
                        ev.engine = inst.engine
                        ev.sync_info = mybir.SyncInfo(
                            on_wait=extra[i:i + 2], on_update=[])
                        nc.register_instruction(ev)
                        insts.insert(idx, ev)
                        idx += 1
                idx += 1
    return n_new


def _host_inputs(hidden_states, position_ids, Wq, Wk, Wv, Wo):
    """Build the 8 per-core input maps."""
    hs = np.asarray(hidden_states, dtype=np.float32)
    pos = np.asarray(position_ids)
    Wq = np.asarray(Wq, dtype=np.float32)
    Wk = np.asarray(Wk, dtype=np.float32)
    Wv = np.asarray(Wv, dtype=np.float32)
    Wo = np.asarray(Wo, dtype=np.float32)
    b, s, h = hs.shape
    qc = h // TP
    kc = (NUM_KV_HEADS * D) // TP
    bf = ml_dtypes.bfloat16

    # rope tables per batch, feature-major, sin sign-folded for rotate_half
    inv_freq = 1.0 / (ROPE_THETA ** (np.arange(0, D, 2, dtype=np.float32) / D))
    maps = []
    i_idx = np.arange(128)[:, None]
    c_idx = np.arange(128)[None, :]
    diagb = np.where(c_idx >= i_idx, 0.0, MASK_VAL).astype(np.float32)

    for c in range(DP * TP):
        bb, r = c // TP, c % TP
        t = pos[bb].astype(np.float64)  # [s]
        ang = t[None, :] * np.concatenate([inv_freq, inv_freq])[:, None]  # [128, s]
        cosb = np.cos(ang).astype(np.float32)
        sinb = np.sin(ang).astype(np.float32)
        sinb[0:64, :] *= -1.0  # rotate_half sign fold
        maps.append({
            "xt": np.ascontiguousarray(hs[bb].T).astype(bf),
            "wq": np.ascontiguousarray(Wq[:, r * qc:(r + 1) * qc]).astype(bf),
            "wk": np.ascontiguousarray(Wk[:, r * kc:(r + 1) * kc]).astype(bf),
            "wv": np.ascontiguousarray(Wv[:, r * kc:(r + 1) * kc]).astype(bf),
            "wo": np.ascontiguousarray(Wo[r * qc:(r + 1) * qc, :]).astype(bf),
            "cosb": cosb.astype(bf),
            "sinb": sinb.astype(bf),
            "diagb": diagb,
        })
    return maps


_NC_CACHE = {}


def _get_nc():
    if "nc" not in _NC_CACHE:
        _NC_CACHE["nc"] = build_nc()
    return _NC_CACHE["nc"]


def kernel(hidden_states, position_ids, Wq, Wk, Wv, Wo, _results_hook=None):
    from concourse.bass_utils import run_bass_kernel_spmd

    maps = _host_inputs(hidden_states, position_ids, Wq, Wk, Wv, Wo)
    nc = _get_nc()
    res = run_bass_kernel_spmd(nc, maps, list(range(DP * TP)))
    if _results_hook is not None:
        _results_hook(res)
    b, s, h = np.asarray(hidden_states).shape
    out = np.zeros((b, s, h), dtype=np.float32)
    for c in range(DP * TP):
        bb = c // TP
        out[bb] += res.results[c]["outp"].T
    return out


if __name__ == "__main__":
    # smoke: build the full-size program and print instruction counts
    nc = build_nc()
    print("built ok")


# revision 13
# speedup vs baseline: 1.2236x; 1.1680x over previous
"""Llama GQA attention (b=2, s=2048, h=4096, 32 Q heads / 8 KV heads, rope)
as a Bass/Tile kernel for 8 Trainium2 NeuronCores.

Sharding: data-parallel over batch (2) x tensor-parallel over heads (4).
Core c = (b, r), b = c // 4, r = c % 4 handles batch b with Q heads
[8r, 8r+8) and KV heads [2r, 2r+2).  Wq/Wk/Wv column-sharded, Wo
row-sharded; per-core output is a partial sum over the TP group which the
host reduces (fp32 adds).

On-core dataflow (all activations feature-major, i.e. transposed):
  XT [H, T] is loaded ONCE into SBUF (resident) and swept three times
  with weights streaming: K sweep, V sweep (token-major, i-outer so it
  pipelines against K's rope evictions), Q sweeps (one head per sweep,
  alternating PSUM rings).  RoPE is applied on PSUM eviction.

  Attention runs per 512-wide q-tile with a software pipeline over heads:
  at step h the PE emits S^T(h) tiles while the row-sum + AV matmuls of
  head h-1 consume the exp'd tiles, so the scalar engine's exp stream
  (the slow stage) is never on the PE's critical path.  Causal masking
  uses a single [128,128] additive diagonal block; fully-masked 128-col
  sub-blocks of band tiles are skipped via partial-width matmuls/exps.
  Softmax is max-free; 1/rowsum via reciprocal_approx_fast, broadcast
  through a PE outer product.

  The Wo projection is interleaved after q-tiles 1 and 3 (all heads'
  O^T columns for those tiles are complete), which keeps the PE warm and
  amortizes weight loads over two 512-wide rhs tiles.
"""

import math
import sys

import numpy as np

for _p in ("/opt/trn_rl_repo",):
    if _p not in sys.path:
        sys.path.insert(0, _p)

import ml_dtypes  # noqa: E402

import concourse.bass as bass  # noqa: E402
import concourse.mybir as mybir  # noqa: E402
import concourse.tile as tile  # noqa: E402
from concourse.alu_op_type import AluOpType  # noqa: E402

F32 = mybir.dt.float32
BF16 = mybir.dt.bfloat16
AF = mybir.ActivationFunctionType

# full problem constants
B, S, H = 2, 2048, 4096
NUM_HEADS, NUM_KV_HEADS, D = 32, 8, 128
ROPE_THETA = 10000.0
TP, DP = 4, 2
MASK_VAL = -30000.0


def build_nc(T=S, HID=H, NQL=NUM_HEADS // TP, NKVL=NUM_KV_HEADS // TP,
             HOUT=H, TQ=512):
    """One-core SPMD program.  T tokens, HID hidden, NQL local Q heads,
    NKVL local KV heads, HOUT output features, TQ q-tile width."""
    assert T % TQ == 0 and TQ % 128 == 0 and HID % 128 == 0
    GRP = NQL // NKVL            # q heads per kv head
    QC = NQL * D                 # local q columns
    KC = NKVL * D                # local kv columns
    KX = HID // 128              # contraction chunks for projections
    NTOK = T // TQ               # token tiles of width TQ
    NT128 = T // 128             # token tiles of width 128
    NKT = TQ // 128              # 128-wide k tiles per q tile
    inv_sqrt_d = 1.0 / math.sqrt(D)

    nc = bass.Bass()
    xt = nc.dram_tensor("xt", [HID, T], BF16, kind="ExternalInput")
    wq = nc.dram_tensor("wq", [HID, QC], BF16, kind="ExternalInput")
    wk = nc.dram_tensor("wk", [HID, KC], BF16, kind="ExternalInput")
    wv = nc.dram_tensor("wv", [HID, KC], BF16, kind="ExternalInput")
    wo = nc.dram_tensor("wo", [QC, HOUT], BF16, kind="ExternalInput")
    cosb = nc.dram_tensor("cosb", [128, T], BF16, kind="ExternalInput")
    sinb = nc.dram_tensor("sinb", [128, T], BF16, kind="ExternalInput")  # sign-folded
    diagb = nc.dram_tensor("diagb", [128, 128], F32, kind="ExternalInput")
    outp = nc.dram_tensor("outp", [HOUT, T], F32, kind="ExternalOutput")

    with tile.TileContext(nc) as tc:
        with (
            tc.tile_pool(name="resident", bufs=1) as res,
            tc.tile_pool(name="const", bufs=1) as const,
        ):
            # resident SBUF arrays (live across the whole kernel)
            qt = [res.tile([128, T], BF16, tag=f"qt{h}", name=f"qt{h}") for h in range(NQL)]
            kt = [res.tile([128, T], BF16, tag=f"kt{h}", name=f"kt{h}") for h in range(NKVL)]
            vt = [res.tile([128, KC], BF16, tag=f"v{t}", name=f"v{t}") for t in range(NT128)]
            cos_sb = res.tile([128, T], BF16, tag="cos")
            sin_sb = res.tile([128, T], BF16, tag="sin")
            diag_sb = res.tile([128, 128], F32, tag="diag")
            ones_col = const.tile([128, 1], BF16)
            ones_row = const.tile([1, 128], BF16)
            nc.vector.memset(ones_col[:], 1.0)
            nc.vector.memset(ones_row[:], 1.0)

            # ---------------- phase 1: projections ----------------
            with (
                tc.tile_pool(name="xres", bufs=1) as xres,
                tc.tile_pool(name="wt", bufs=1) as wpool,
                tc.tile_pool(name="rope_tmp", bufs=1) as rpool,
            ):
                # resident XT: one HBM read for all sweeps (DMAs emitted
                # inside the V sweep, interleaved with the wv chunks, so the
                # first matmuls aren't queued behind the whole 16MB load)
                xts = [xres.tile([128, T], BF16, tag=f"x{k}", name=f"x{k}")
                       for k in range(KX)]

                def rope_evict(ps, dst_ap, tok0):
                    """dst = ps*cos + rot_half(ps)*sin  (sin sign-folded)."""
                    cw = cos_sb[:, tok0:tok0 + TQ]
                    sw = sin_sb[:, tok0:tok0 + TQ]
                    r = rpool.tile([128, TQ], F32, tag="rot", bufs=2, name="rot")
                    nc.scalar.copy(r[0:64, :], ps[64:128, :])
                    nc.scalar.copy(r[64:128, :], ps[0:64, :])
                    t1 = rpool.tile([128, TQ], F32, tag="t1", bufs=2, name="t1")
                    nc.vector.tensor_tensor(t1[:], ps[:], cw, op=AluOpType.mult)
                    nc.vector.tensor_tensor(r[:], r[:], sw, op=AluOpType.mult)
                    nc.vector.tensor_tensor(dst_ap, t1[:], r[:], op=AluOpType.add)

                # V sweep first (no prior-eviction PSUM waits): token-major,
                # k-outer, two groups of 8 banks
                with tc.tile_pool(name="pv", bufs=1, space="PSUM") as pv:
                    for g in range(NT128 // 8):
                        psv = [pv.tile([128, KC], F32, tag=f"pv{i}", name=f"psv{i}")
                               for i in range(8)]
                        for k in range(KX):
                            if g == 0:
                                nc.sync.dma_start(out=xts[k][:],
                                                  in_=xt[k * 128:(k + 1) * 128, :])
                            wv_sb = wpool.tile([128, KC], BF16, tag="wv", bufs=6,
                                               name="wv_sb")
                            nc.sync.dma_start(out=wv_sb[:],
                                              in_=wv[k * 128:(k + 1) * 128, :])
                            for i in range(8):
                                t0 = (g * 8 + i) * 128
                                nc.tensor.matmul(
                                    psv[i][:], lhsT=xts[k][:, t0:t0 + 128],
                                    rhs=wv_sb[:],
                                    start=(k == 0), stop=(k == KX - 1))
                        if g == 0:
                            nc.sync.dma_start(out=cos_sb[:], in_=cosb[:])
                            nc.sync.dma_start(out=sin_sb[:], in_=sinb[:])
                            nc.sync.dma_start(out=diag_sb[:], in_=diagb[:])
                        for i in range(8):
                            nc.scalar.copy(vt[g * 8 + i][:], psv[i][:])

                # K sweep: one head per pass (4 banks), so pass i's rope
                # evictions drain behind pass i+1's matmuls
                with tc.tile_pool(name="pk", bufs=1, space="PSUM") as pk:
                    for i in range(NKVL):
                        psk = [pk.tile([128, TQ], F32, tag=f"pk{i % 2}{j}",
                                       name=f"pk{j}") for j in range(NTOK)]
                        for k in range(KX):
                            wk_sb = wpool.tile([128, KC], BF16, tag="wk", bufs=6,
                                               name="wk_sb")
                            nc.sync.dma_start(out=wk_sb[:],
                                              in_=wk[k * 128:(k + 1) * 128, :])
                            for j in range(NTOK):
                                nc.tensor.matmul(
                                    psk[j][:],
                                    lhsT=wk_sb[:, i * 128:(i + 1) * 128],
                                    rhs=xts[k][:, j * TQ:(j + 1) * TQ],
                                    start=(k == 0), stop=(k == KX - 1))
                        for j in range(NTOK):
                            rope_evict(psk[j], kt[i][:, j * TQ:(j + 1) * TQ], j * TQ)

                # Q sweeps: one head per sweep, 4 banks, alternating rings
                with tc.tile_pool(name="pq", bufs=1, space="PSUM") as pq:
                    for h in range(NQL):
                        psq = [pq.tile([128, TQ], F32, tag=f"pq{j}", bufs=2,
                                       name=f"psq{j}") for j in range(NTOK)]
                        for k in range(KX):
                            wq_sb = wpool.tile([128, 128], BF16, tag="wq", bufs=8,
                                               name="wq_sb")
                            nc.sync.dma_start(
                                out=wq_sb[:],
                                in_=wq[k * 128:(k + 1) * 128, h * 128:(h + 1) * 128])
                            for j in range(NTOK):
                                nc.tensor.matmul(
                                    psq[j][:], lhsT=wq_sb[:],
                                    rhs=xts[k][:, j * TQ:(j + 1) * TQ],
                                    start=(k == 0), stop=(k == KX - 1))
                        for j in range(NTOK):
                            rope_evict(psq[j], qt[h][:, j * TQ:(j + 1) * TQ], j * TQ)

            # -------- phase 2+3: attention with interleaved Wo --------
            CT = QC // 128  # Wo contraction chunks (== NQL)
            with (
                tc.tile_pool(name="ot_pool", bufs=1) as otpool,
                tc.tile_pool(name="wo_sb", bufs=1) as wopool,
                tc.tile_pool(name="es", bufs=1) as epool,
                tc.tile_pool(name="at_small", bufs=1) as spool,
                tc.tile_pool(name="ob", bufs=1) as obpool,
            ):
                ot = [otpool.tile([128, T], BF16, tag=f"ot{h}", name=f"ot{h}")
                      for h in range(NQL)]
                wos = [wopool.tile([128, HOUT], BF16, tag=f"wo{c}", name=f"wos{c}")
                       for c in range(CT)]
                for c in range(CT):
                    nc.sync.dma_start(out=wos[c][:], in_=wo[c * 128:(c + 1) * 128, :])

                for qi in range(NTOK):
                    nk = (qi + 1) * NKT
                    q0 = qi * TQ
                    with (
                        tc.tile_pool(name="ps_s", bufs=1, space="PSUM") as psum_s,
                        tc.tile_pool(name="ps_o", bufs=1, space="PSUM") as psum_o,
                        tc.tile_pool(name="ps_n", bufs=1, space="PSUM") as psum_n,
                    ):
                        es = {}
                        ctx = {}
                        pending = []

                        def col0(ki):
                            j = ki - qi * NKT
                            return 128 * j if j > 0 else 0

                        def flush_pending():
                            # 1/rowsum = exp(-ln(rowsum)), both on ACT: avoids
                            # the 4us iterative-divide DVE reciprocal
                            hp, ps_o, ps_n = pending.pop(0)
                            lr = spool.tile([1, TQ], F32, tag="lr", bufs=2,
                                            name="lr")
                            nc.scalar.activation(lr[:], ps_n[0:1, :], AF.Ln)
                            rcb = spool.tile([1, TQ], BF16, tag="rcb", bufs=2,
                                             name="rcb")
                            nc.scalar.activation(rcb[:], lr[:], AF.Exp, scale=-1.0)
                            ps_b = psum_s.tile([128, TQ], F32, tag="s", bufs=4,
                                               name="ps_b")
                            nc.tensor.matmul(ps_b[:], lhsT=ones_row[:],
                                             rhs=rcb[:], start=True, stop=True)
                            bc = spool.tile([128, TQ], BF16, tag="bc", bufs=2,
                                            name="bc")
                            nc.vector.tensor_copy(bc[:], ps_b[:])
                            nc.vector.tensor_tensor(
                                ot[hp][:, q0:q0 + TQ], ps_o[:], bc[:],
                                op=AluOpType.mult)

                        for hr in range(NQL + 1):
                            h = hr if hr < NQL else None          # producer
                            hc = hr - 1 if hr >= 1 else None      # consumer
                            if hc is not None:
                                ps_o = psum_o.tile([128, TQ], F32, tag="o", bufs=2,
                                                   name="ps_o")
                                ps_n = psum_n.tile([1, TQ], F32, tag="n", bufs=2,
                                                   name="ps_n")
                                ctx[hc] = (ps_o, ps_n)
                            for ki in range(nk):
                                c0 = col0(ki)
                                if h is not None:
                                    kvh = h // GRP
                                    ps_s = psum_s.tile([128, TQ], F32, tag="s",
                                                       bufs=4, name="ps_s")
                                    nc.tensor.matmul(
                                        ps_s[:, c0:TQ],
                                        lhsT=kt[kvh][:, ki * 128:(ki + 1) * 128],
                                        rhs=qt[h][:, q0 + c0:q0 + TQ],
                                        start=True, stop=True)
                                    if ki >= qi * NKT:  # diagonal block mask
                                        nc.vector.tensor_tensor(
                                            ps_s[:, c0:c0 + 128], ps_s[:, c0:c0 + 128],
                                            diag_sb[:], op=AluOpType.add)
                                    e = epool.tile([128, TQ], BF16,
                                                   tag=f"e{h % 2}_{ki}", name="e")
                                    nc.scalar.activation(e[:, c0:TQ], ps_s[:, c0:TQ],
                                                         AF.Exp, scale=inv_sqrt_d)
                                    es[(h % 2, ki)] = e
                                if hc is not None:
                                    kvc = hc // GRP
                                    ps_o, ps_n = ctx[hc]
                                    ec = es[(hc % 2, ki)]
                                    nc.tensor.matmul(
                                        ps_n[0:1, c0:TQ], lhsT=ones_col[:],
                                        rhs=ec[:, c0:TQ],
                                        start=(ki == 0), stop=(ki == nk - 1))
                                    nc.tensor.matmul(
                                        ps_o[:, c0:TQ],
                                        lhsT=vt[ki][:, kvc * D:(kvc + 1) * D],
                                        rhs=ec[:, c0:TQ],
                                        start=(ki == 0), stop=(ki == nk - 1))
                            if hc is not None:
                                pending.append((hc, *ctx.pop(hc)))
                                if len(pending) > 1:
                                    flush_pending()
                        while pending:
                            flush_pending()

                # Wo projection as one dense phase: 4 rhs tiles per weight
                # load, no cross-engine waits -> LDWEIGHTS stays hidden
                with tc.tile_pool(name="po", bufs=1, space="PSUM") as pop:
                    for ni in range(HOUT // 128):
                        ps = [pop.tile([128, TQ], F32, tag=f"po{j}", bufs=2,
                                       name=f"po{j}") for j in range(NTOK)]
                        for c in range(CT):
                            for j in range(NTOK):
                                nc.tensor.matmul(
                                    ps[j][:],
                                    lhsT=wos[c][:, ni * 128:(ni + 1) * 128],
                                    rhs=ot[c][:, j * TQ:(j + 1) * TQ],
                                    start=(c == 0), stop=(c == CT - 1))
                        for j in range(NTOK):
                            ob = obpool.tile([128, TQ], F32, tag="ob",
                                             bufs=4, name="ob")
                            nc.scalar.copy(ob[:], ps[j][:])
                            nc.sync.dma_start(
                                out=outp[ni * 128:(ni + 1) * 128,
                                         j * TQ:(j + 1) * TQ],
                                in_=ob[:])
    legalize_wait_counts(nc)
    return nc


def legalize_wait_counts(nc):
    """walrus DIRECT2D descriptors accept a single sync-wait; Tile can emit
    more (data wait + queue-head wait).  Hoist excess waits onto
    EventSemaphore instructions inserted just before, on the same engine."""
    n_new = 0
    for f in nc.m.functions:
        for blk in f.blocks:
            idx = 0
            insts = blk.instructions
            while idx < len(insts):
                inst = insts[idx]
                si = getattr(inst, "sync_info", None)
                cap = 2 if isinstance(inst, mybir.InstEventSemaphore) else 1
                waits = list(si.on_wait) if si is not None and si.on_wait else []
                if len(waits) > cap:
                    keep, extra = waits[-cap:], waits[:-cap]
                    si.on_wait = keep
                    for i in range(0, len(extra), 2):
                        ev = mybir.InstEventSemaphore(
                            name=f"waitsplit_{n_new}", ins=[], outs=[])
                        n_new += 1
                        ev.engine = inst.engine
                        ev.sync_info = mybir.SyncInfo(
                            on_wait=extra[i:i + 2], on_update=[])
                        nc.register_instruction(ev)
                        insts.insert(idx, ev)
                        idx += 1
                idx += 1
    return n_new


def _host_inputs(hidden_states, position_ids, Wq, Wk, Wv, Wo):
    """Build the 8 per-core input maps."""
    hs = np.asarray(hidden_states, dtype=np.float32)
    pos = np.asarray(position_ids)
    Wq = np.asarray(Wq, dtype=np.float32)
    Wk = np.asarray(Wk, dtype=np.float32)
    Wv = np.asarray(Wv, dtype=np.float32)
    Wo = np.asarray(Wo, dtype=np.float32)
    b, s, h = hs.shape
    qc = h // TP
    kc = (NUM_KV_HEADS * D) // TP
    bf = ml_dtypes.bfloat16

    # rope tables per batch, feature-major, sin sign-folded for rotate_half
    inv_freq = 1.0 / (ROPE_THETA ** (np.arange(0, D, 2, dtype=np.float32) / D))
    maps = []
    i_idx = np.arange(128)[:, None]
    c_idx = np.arange(128)[None, :]
    diagb = np.where(c_idx >= i_idx, 0.0, MASK_VAL).astype(np.float32)

    for c in range(DP * TP):
        bb, r = c // TP, c % TP
        t = pos[bb].astype(np.float64)  # [s]
        ang = t[None, :] * np.concatenate([inv_freq, inv_freq])[:, None]  # [128, s]
        cosb = np.cos(ang).astype(np.float32)
        sinb = np.sin(ang).astype(np.float32)
        sinb[0:64, :] *= -1.0  # rotate_half sign fold
        maps.append({
            "xt": np.ascontiguousarray(hs[bb].T).astype(bf),
            "wq": np.ascontiguousarray(Wq[:, r * qc:(r + 1) * qc]).astype(bf),
            "wk": np.ascontiguousarray(Wk[:, r * kc:(r + 1) * kc]).astype(bf),
            "wv": np.ascontiguousarray(Wv[:, r * kc:(r + 1) * kc]).astype(bf),
            "wo": np.ascontiguousarray(Wo[r * qc:(r + 1) * qc, :]).astype(bf),
            "cosb": cosb.astype(bf),
            "sinb": sinb.astype(bf),
            "diagb": diagb,
        })
    return maps


_NC_CACHE = {}


def _get_nc():
    if "nc" not in _NC_CACHE:
        _NC_CACHE["nc"] = build_nc()
    return _NC_CACHE["nc"]


def kernel(hidden_states, position_ids, Wq, Wk, Wv, Wo, _results_hook=None):
    from concourse.bass_utils import run_bass_kernel_spmd

    maps = _host_inputs(hidden_states, position_ids, Wq, Wk, Wv, Wo)
    nc = _get_nc()
    res = run_bass_kernel_spmd(nc, maps, list(range(DP * TP)))
    if _results_hook is not None:
        _results_hook(res)
    b, s, h = np.asarray(hidden_states).shape
    out = np.zeros((b, s, h), dtype=np.float32)
    for c in range(DP * TP):
        bb = c // TP
        out[bb] += res.results[c]["outp"].T
    return out


if __name__ == "__main__":
    # smoke: build the full-size program and print instruction counts
    nc = build_nc()
    print("built ok")


# revision 19
# speedup vs baseline: 1.4104x; 1.1526x over previous
"""Llama GQA attention (b=2, s=2048, h=4096, 32 Q heads / 8 KV heads, rope)
as a Bass/Tile kernel for 8 Trainium2 NeuronCores.

Sharding: data-parallel over batch (2) x tensor-parallel over heads (4).
Core c = (b, r), b = c // 4, r = c % 4 handles batch b with Q heads
[8r, 8r+8) and KV heads [2r, 2r+2).  Wq/Wk/Wv column-sharded, Wo
row-sharded; per-core output is a partial sum over the TP group which the
host reduces (fp32 adds).

On-core dataflow (all activations feature-major, i.e. transposed):
  XT [H, T] is loaded ONCE into SBUF (resident) and swept three times
  with weights streaming: K sweep, V sweep (token-major, i-outer so it
  pipelines against K's rope evictions), Q sweeps (one head per sweep,
  alternating PSUM rings).  RoPE is applied on PSUM eviction.

  Attention runs per 512-wide q-tile with a software pipeline over heads:
  at step h the PE emits S^T(h) tiles while the row-sum + AV matmuls of
  head h-1 consume the exp'd tiles, so the scalar engine's exp stream
  (the slow stage) is never on the PE's critical path.  Causal masking
  uses a single [128,128] additive diagonal block; fully-masked 128-col
  sub-blocks of band tiles are skipped via partial-width matmuls/exps.
  Softmax is max-free; 1/rowsum via reciprocal_approx_fast, broadcast
  through a PE outer product.

  The Wo projection is interleaved after q-tiles 1 and 3 (all heads'
  O^T columns for those tiles are complete), which keeps the PE warm and
  amortizes weight loads over two 512-wide rhs tiles.
"""

import math
import sys

import numpy as np

for _p in ("/opt/trn_rl_repo",):
    if _p not in sys.path:
        sys.path.insert(0, _p)

import ml_dtypes  # noqa: E402

import concourse.bass as bass  # noqa: E402
import concourse.mybir as mybir  # noqa: E402
import concourse.tile as tile  # noqa: E402
from concourse.alu_op_type import AluOpType  # noqa: E402

F32 = mybir.dt.float32
BF16 = mybir.dt.bfloat16
AF = mybir.ActivationFunctionType

# full problem constants
B, S, H = 2, 2048, 4096
NUM_HEADS, NUM_KV_HEADS, D = 32, 8, 128
ROPE_THETA = 10000.0
TP, DP = 4, 2
MASK_VAL = -30000.0


def build_nc(T=S, HID=H, NQL=NUM_HEADS // TP, NKVL=NUM_KV_HEADS // TP,
             HOUT=H, TQ=512):
    """One-core SPMD program.  T tokens, HID hidden, NQL local Q heads,
    NKVL local KV heads, HOUT output features, TQ q-tile width."""
    assert T % TQ == 0 and TQ % 128 == 0 and HID % 128 == 0
    GRP = NQL // NKVL            # q heads per kv head
    QC = NQL * D                 # local q columns
    KC = NKVL * D                # local kv columns
    KX = HID // 128              # contraction chunks for projections
    NTOK = T // TQ               # token tiles of width TQ
    NT128 = T // 128             # token tiles of width 128
    NKT = TQ // 128              # 128-wide k tiles per q tile
    inv_sqrt_d = 1.0 / math.sqrt(D)

    nc = bass.Bass()
    xt = nc.dram_tensor("xt", [HID, T], BF16, kind="ExternalInput")
    wq = nc.dram_tensor("wq", [HID, QC], BF16, kind="ExternalInput")
    wk = nc.dram_tensor("wk", [HID, KC], BF16, kind="ExternalInput")
    wv = nc.dram_tensor("wv", [HID, KC], BF16, kind="ExternalInput")
    wo = nc.dram_tensor("wo", [QC, HOUT], BF16, kind="ExternalInput")
    cosb = nc.dram_tensor("cosb", [128, T], BF16, kind="ExternalInput")
    sinb = nc.dram_tensor("sinb", [128, T], BF16, kind="ExternalInput")  # sign-folded
    diagb = nc.dram_tensor("diagb", [128, 128], F32, kind="ExternalInput")
    outp = nc.dram_tensor("outp", [HOUT, T], F32, kind="ExternalOutput")

    with tile.TileContext(nc) as tc:
        with (
            tc.tile_pool(name="resident", bufs=1) as res,
            tc.tile_pool(name="const", bufs=1) as const,
        ):
            # resident SBUF arrays (live across the whole kernel)
            qt = [res.tile([128, T], BF16, tag=f"qt{h}", name=f"qt{h}") for h in range(NQL)]
            kt = [res.tile([128, T], BF16, tag=f"kt{h}", name=f"kt{h}") for h in range(NKVL)]
            vt = [res.tile([128, KC], BF16, tag=f"v{t}", name=f"v{t}") for t in range(NT128)]
            cos_sb = res.tile([128, T], BF16, tag="cos")
            sin_sb = res.tile([128, T], BF16, tag="sin")
            diag_sb = res.tile([128, 128], F32, tag="diag")
            ones_col = const.tile([128, 1], BF16)
            ones_row = const.tile([1, 128], BF16)
            nc.vector.memset(ones_col[:], 1.0)
            nc.vector.memset(ones_row[:], 1.0)

            # ---------------- phase 1: projections ----------------
            with (
                tc.tile_pool(name="xres", bufs=1) as xres,
                tc.tile_pool(name="wt", bufs=1) as wpool,
                tc.tile_pool(name="rope_tmp", bufs=1) as rpool,
            ):
                # resident XT: one HBM read for all sweeps (DMAs emitted
                # inside the V sweep, interleaved with the wv chunks, so the
                # first matmuls aren't queued behind the whole 16MB load)
                xts = [xres.tile([128, T], BF16, tag=f"x{k}", name=f"x{k}")
                       for k in range(KX)]

                def rope_evict(ps, dst_ap, tok0):
                    """dst = ps*cos + rot_half(ps)*sin  (sin sign-folded)."""
                    cw = cos_sb[:, tok0:tok0 + TQ]
                    sw = sin_sb[:, tok0:tok0 + TQ]
                    r = rpool.tile([128, TQ], F32, tag="rot", bufs=2, name="rot")
                    nc.scalar.copy(r[0:64, :], ps[64:128, :])
                    nc.scalar.copy(r[64:128, :], ps[0:64, :])
                    t1 = rpool.tile([128, TQ], F32, tag="t1", bufs=2, name="t1")
                    nc.vector.tensor_tensor(t1[:], ps[:], cw, op=AluOpType.mult)
                    nc.vector.tensor_tensor(r[:], r[:], sw, op=AluOpType.mult)
                    nc.vector.tensor_tensor(dst_ap, t1[:], r[:], op=AluOpType.add)

                # V sweep first (no prior-eviction PSUM waits): token-major,
                # k-outer, two groups of 8 banks
                with tc.tile_pool(name="pv", bufs=1, space="PSUM") as pv:
                    for g in range(NT128 // 8):
                        psv = [pv.tile([128, KC], F32, tag=f"pv{i}", name=f"psv{i}")
                               for i in range(8)]
                        for k in range(KX):
                            if g == 0:
                                nc.sync.dma_start(out=xts[k][:],
                                                  in_=xt[k * 128:(k + 1) * 128, :])
                            wv_sb = wpool.tile([128, KC], BF16, tag="wv", bufs=6,
                                               name="wv_sb")
                            nc.sync.dma_start(out=wv_sb[:],
                                              in_=wv[k * 128:(k + 1) * 128, :])
                            for i in range(8):
                                t0 = (g * 8 + i) * 128
                                nc.tensor.matmul(
                                    psv[i][:], lhsT=xts[k][:, t0:t0 + 128],
                                    rhs=wv_sb[:],
                                    start=(k == 0), stop=(k == KX - 1))
                        if g == 0:
                            nc.sync.dma_start(out=cos_sb[:], in_=cosb[:])
                            nc.sync.dma_start(out=sin_sb[:], in_=sinb[:])
                            nc.sync.dma_start(out=diag_sb[:], in_=diagb[:])
                        for i in range(8):
                            nc.scalar.copy(vt[g * 8 + i][:], psv[i][:])

                # K sweep: one head per pass (4 banks), so pass i's rope
                # evictions drain behind pass i+1's matmuls
                with tc.tile_pool(name="pk", bufs=1, space="PSUM") as pk:
                    for i in range(NKVL):
                        psk = [pk.tile([128, TQ], F32, tag=f"pk{i % 2}{j}",
                                       name=f"pk{j}") for j in range(NTOK)]
                        for k in range(KX):
                            wk_sb = wpool.tile([128, KC], BF16, tag="wk", bufs=6,
                                               name="wk_sb")
                            nc.sync.dma_start(out=wk_sb[:],
                                              in_=wk[k * 128:(k + 1) * 128, :])
                            for j in range(NTOK):
                                nc.tensor.matmul(
                                    psk[j][:],
                                    lhsT=wk_sb[:, i * 128:(i + 1) * 128],
                                    rhs=xts[k][:, j * TQ:(j + 1) * TQ],
                                    start=(k == 0), stop=(k == KX - 1))
                        for j in range(NTOK):
                            rope_evict(psk[j], kt[i][:, j * TQ:(j + 1) * TQ], j * TQ)

                # Q sweeps: one head per sweep, 4 banks, alternating rings
                with tc.tile_pool(name="pq", bufs=1, space="PSUM") as pq:
                    for h in range(NQL):
                        psq = [pq.tile([128, TQ], F32, tag=f"pq{j}", bufs=2,
                                       name=f"psq{j}") for j in range(NTOK)]
                        for k in range(KX):
                            wq_sb = wpool.tile([128, 128], BF16, tag="wq", bufs=8,
                                               name="wq_sb")
                            nc.sync.dma_start(
                                out=wq_sb[:],
                                in_=wq[k * 128:(k + 1) * 128, h * 128:(h + 1) * 128])
                            for j in range(NTOK):
                                nc.tensor.matmul(
                                    psq[j][:], lhsT=wq_sb[:],
                                    rhs=xts[k][:, j * TQ:(j + 1) * TQ],
                                    start=(k == 0), stop=(k == KX - 1))
                        for j in range(NTOK):
                            rope_evict(psq[j], qt[h][:, j * TQ:(j + 1) * TQ], j * TQ)

            # -------- phase 2: attention, head-pair pipelined --------
            # Producer emits S^T for a pair of heads sharing each kt weight
            # load, as [128, 1024] ki-pair PSUM tiles (one exp per pair).
            # Consumer (previous pair) runs AV + row-sums, vt loads shared.
            # All 8 row-sum accumulators live in ONE PSUM bank as M=1
            # matmuls into distinct 32-partition column groups.
            CT = QC // 128  # Wo contraction chunks (== NQL)
            NPAIR = NQL // 2
            with (
                tc.tile_pool(name="ot_pool", bufs=1) as otpool,
                tc.tile_pool(name="es", bufs=1) as epool,
                tc.tile_pool(name="at_small", bufs=1) as spool,
                tc.tile_pool(name="ob", bufs=1) as obpool,
            ):
                ot = [otpool.tile([128, T], BF16, tag=f"ot{h}", name=f"ot{h}")
                      for h in range(NQL)]

                for qi in range(NTOK):
                    nk = (qi + 1) * NKT
                    npr = nk // 2  # ki pairs
                    q0 = qi * TQ
                    with (
                        tc.tile_pool(name="ps_s", bufs=1, space="PSUM") as psum_s,
                        tc.tile_pool(name="ps_o", bufs=1, space="PSUM") as psum_o,
                        tc.tile_pool(name="ps_n", bufs=1, space="PSUM") as psum_n,
                    ):
                        es = {}
                        pending = []

                        def col0(ki):
                            j = ki - qi * NKT
                            return 128 * j if j > 0 else 0

                        def flush_pending():
                            # 1/rowsum = exp(-ln(rowsum)) on ACT (the DVE
                            # reciprocal is an 8x-repeat iterative divide)
                            hp, po_, pn_ = pending.pop(0)
                            lr = spool.tile([1, TQ], F32, tag="lr", bufs=2,
                                            name="lr")
                            nc.scalar.activation(lr[:], pn_[0:1, :], AF.Ln)
                            rcb = spool.tile([1, TQ], BF16, tag="rcb", bufs=2,
                                             name="rcb")
                            nc.scalar.activation(rcb[:], lr[:], AF.Exp, scale=-1.0)
                            ps_b = psum_s.tile([128, 2 * TQ], F32, tag="sa",
                                               name="ps_b")
                            nc.tensor.matmul(ps_b[:, 0:TQ], lhsT=ones_row[:],
                                             rhs=rcb[:], start=True, stop=True)
                            bc = spool.tile([128, TQ], BF16, tag="bc", bufs=2,
                                            name="bc")
                            nc.vector.tensor_copy(bc[:], ps_b[:, 0:TQ])
                            nc.vector.tensor_tensor(
                                ot[hp][:, q0:q0 + TQ], po_[:], bc[:],
                                op=AluOpType.mult)

                        for rr in range(NPAIR + 2):
                            while pending:
                                flush_pending()
                            prod = rr if rr < NPAIR else None
                            cons = rr - 1 if 1 <= rr <= NPAIR else None
                            if cons is not None:
                                ha, hb = 2 * cons, 2 * cons + 1
                                kvc = ha // GRP
                                po_a = psum_o.tile([128, TQ], F32, tag="o",
                                                   bufs=2, name="po_a")
                                po_b = psum_o.tile([128, TQ], F32, tag="o",
                                                   bufs=2, name="po_b")
                                pn_a = psum_n.tile([1, TQ], F32, tag="n",
                                                   bufs=2, name="pn_a")
                                pn_b = psum_n.tile([1, TQ], F32, tag="n",
                                                   bufs=2, name="pn_b")
                            if prod is not None:
                                pa_, pb_ = 2 * prod, 2 * prod + 1
                                kvp = pa_ // GRP
                            for p in range(npr):
                                if prod is not None:
                                    s2a = psum_s.tile([128, 2 * TQ], F32, tag="sa",
                                                      name="s2a")
                                    s2b = psum_s.tile([128, 2 * TQ], F32, tag="sb",
                                                      name="s2b")
                                    for sub in range(2):
                                        ki = 2 * p + sub
                                        c0 = col0(ki)
                                        for hh, s2 in ((pa_, s2a), (pb_, s2b)):
                                            nc.tensor.matmul(
                                                s2[:, sub * TQ + c0:(sub + 1) * TQ],
                                                lhsT=kt[kvp][:, ki * 128:(ki + 1) * 128],
                                                rhs=qt[hh][:, q0 + c0:q0 + TQ],
                                                start=True, stop=True)
                                        if ki >= qi * NKT:  # diagonal block
                                            for s2 in (s2a, s2b):
                                                nc.vector.tensor_tensor(
                                                    s2[:, sub * TQ + c0:
                                                       sub * TQ + c0 + 128],
                                                    s2[:, sub * TQ + c0:
                                                       sub * TQ + c0 + 128],
                                                    diag_sb[:], op=AluOpType.add)
                                    pc0 = col0(2 * p)
                                    for hh, s2 in ((pa_, s2a), (pb_, s2b)):
                                        e = epool.tile(
                                            [128, 2 * TQ], BF16,
                                            tag=f"e{prod % 2}{hh % 2}_{p}", name="e")
                                        nc.scalar.activation(
                                            e[:, pc0:2 * TQ], s2[:, pc0:2 * TQ],
                                            AF.Exp, scale=inv_sqrt_d)
                                        es[(hh, p)] = e
                                if cons is not None:
                                    for sub in range(2):
                                        ki = 2 * p + sub
                                        c0 = col0(ki)
                                        for hh, pn_ in ((ha, pn_a), (hb, pn_b)):
                                            nc.tensor.matmul(
                                                pn_[0:1, c0:TQ],
                                                lhsT=ones_col[:],
                                                rhs=es[(hh, p)][:, sub * TQ + c0:
                                                                (sub + 1) * TQ],
                                                start=(ki == 0),
                                                stop=(ki == nk - 1))
                                        for hh, po_ in ((ha, po_a), (hb, po_b)):
                                            nc.tensor.matmul(
                                                po_[:, c0:TQ],
                                                lhsT=vt[ki][:, kvc * D:(kvc + 1) * D],
                                                rhs=es[(hh, p)][:, sub * TQ + c0:
                                                                (sub + 1) * TQ],
                                                start=(ki == 0),
                                                stop=(ki == nk - 1))
                            if cons is not None:
                                pending.append((ha, po_a, pn_a))
                                pending.append((hb, po_b, pn_b))
                        while pending:
                            flush_pending()

                # -------- phase 3: Wo projection, streamed weight groups ----
                with (
                    tc.tile_pool(name="wog", bufs=1) as wogp,
                    tc.tile_pool(name="po", bufs=1, space="PSUM") as pop,
                ):
                    NG = 8
                    for g0 in range(0, HOUT // 128, NG):
                        wos_g = []
                        for c in range(CT):
                            w = wogp.tile([128, NG * 128], BF16, tag=f"wg{c}",
                                          bufs=2, name=f"wg{c}")
                            nc.sync.dma_start(
                                out=w[:],
                                in_=wo[c * 128:(c + 1) * 128,
                                       g0 * 128:(g0 + NG) * 128])
                            wos_g.append(w)
                        for i in range(NG):
                            ni = g0 + i
                            ps = [pop.tile([128, TQ], F32, tag=f"po{j}", bufs=2,
                                           name=f"po{j}") for j in range(NTOK)]
                            for c in range(CT):
                                for j in range(NTOK):
                                    nc.tensor.matmul(
                                        ps[j][:],
                                        lhsT=wos_g[c][:, i * 128:(i + 1) * 128],
                                        rhs=ot[c][:, j * TQ:(j + 1) * TQ],
                                        start=(c == 0), stop=(c == CT - 1))
                            for j in range(NTOK):
                                ob = obpool.tile([128, TQ], F32, tag="ob",
                                                 bufs=4, name="ob")
                                nc.scalar.copy(ob[:], ps[j][:])
                                nc.sync.dma_start(
                                    out=outp[ni * 128:(ni + 1) * 128,
                                             j * TQ:(j + 1) * TQ],
                                    in_=ob[:])
    legalize_wait_counts(nc)
    return nc


def legalize_wait_counts(nc):
    """walrus DIRECT2D descriptors accept a single sync-wait; Tile can emit
    more (data wait + queue-head wait).  Hoist excess waits onto
    EventSemaphore instructions inserted just before, on the same engine."""
    n_new = 0
    for f in nc.m.functions:
        for blk in f.blocks:
            idx = 0
            insts = blk.instructions
            while idx < len(insts):
                inst = insts[idx]
                si = getattr(inst, "sync_info", None)
                cap = 2 if isinstance(inst, mybir.InstEventSemaphore) else 1
                waits = list(si.on_wait) if si is not None and si.on_wait else []
                if len(waits) > cap:
                    keep, extra = waits[-cap:], waits[:-cap]
                    si.on_wait = keep
                    for i in range(0, len(extra), 2):
                        ev = mybir.InstEventSemaphore(
                            name=f"waitsplit_{n_new}", ins=[], outs=[])
                        n_new += 1
                        ev.engine = inst.engine
                        ev.sync_info = mybir.SyncInfo(
                            on_wait=extra[i:i + 2], on_update=[])
                        nc.register_instruction(ev)
                        insts.insert(idx, ev)
                        idx += 1
                idx += 1
    return n_new


def _host_inputs(hidden_states, position_ids, Wq, Wk, Wv, Wo):
    """Build the 8 per-core input maps."""
    hs = np.asarray(hidden_states, dtype=np.float32)
    pos = np.asarray(position_ids)
    Wq = np.asarray(Wq, dtype=np.float32)
    Wk = np.asarray(Wk, dtype=np.float32)
    Wv = np.asarray(Wv, dtype=np.float32)
    Wo = np.asarray(Wo, dtype=np.float32)
    b, s, h = hs.shape
    qc = h // TP
    kc = (NUM_KV_HEADS * D) // TP
    bf = ml_dtypes.bfloat16

    # rope tables per batch, feature-major, sin sign-folded for rotate_half
    inv_freq = 1.0 / (ROPE_THETA ** (np.arange(0, D, 2, dtype=np.float32) / D))
    maps = []
    i_idx = np.arange(128)[:, None]
    c_idx = np.arange(128)[None, :]
    diagb = np.where(c_idx >= i_idx, 0.0, MASK_VAL).astype(np.float32)

    for c in range(DP * TP):
        bb, r = c // TP, c % TP
        t = pos[bb].astype(np.float64)  # [s]
        ang = t[None, :] * np.concatenate([inv_freq, inv_freq])[:, None]  # [128, s]
        cosb = np.cos(ang).astype(np.float32)
        sinb = np.sin(ang).astype(np.float32)
        sinb[0:64, :] *= -1.0  # rotate_half sign fold
        maps.append({
            "xt": np.ascontiguousarray(hs[bb].T).astype(bf),
            "wq": np.ascontiguousarray(Wq[:, r * qc:(r + 1) * qc]).astype(bf),
            "wk": np.ascontiguousarray(Wk[:, r * kc:(r + 1) * kc]).astype(bf),
            "wv": np.ascontiguousarray(Wv[:, r * kc:(r + 1) * kc]).astype(bf),
            "wo": np.ascontiguousarray(Wo[r * qc:(r + 1) * qc, :]).astype(bf),
            "cosb": cosb.astype(bf),
            "sinb": sinb.astype(bf),
            "diagb": diagb,
        })
    return maps


_NC_CACHE = {}


def _get_nc():
    if "nc" not in _NC_CACHE:
        _NC_CACHE["nc"] = build_nc()
    return _NC_CACHE["nc"]


def kernel(hidden_states, position_ids, Wq, Wk, Wv, Wo, _results_hook=None):
    from concourse.bass_utils import run_bass_kernel_spmd

    maps = _host_inputs(hidden_states, position_ids, Wq, Wk, Wv, Wo)
    nc = _get_nc()
    res = run_bass_kernel_spmd(nc, maps, list(range(DP * TP)))
    if _results_hook is not None:
        _results_hook(res)
    b, s, h = np.asarray(hidden_states).shape
    out = np.zeros((b, s, h), dtype=np.float32)
    for c in range(DP * TP):
        bb = c // TP
        out[bb] += res.results[c]["outp"].T
    return out


if __name__ == "__main__":
    # smoke: build the full-size program and print instruction counts
    nc = build_nc()
    print("built ok")


# revision 22
# speedup vs baseline: 1.4200x; 1.0068x over previous
"""Llama GQA attention (b=2, s=2048, h=4096, 32 Q heads / 8 KV heads, rope)
as a Bass/Tile kernel for 8 Trainium2 NeuronCores.

Sharding: data-parallel over batch (2) x tensor-parallel over heads (4).
Core c = (b, r), b = c // 4, r = c % 4 handles batch b with Q heads
[8r, 8r+8) and KV heads [2r, 2r+2).  Wq/Wk/Wv column-sharded, Wo
row-sharded; per-core output is a partial sum over the TP group which the
host reduces (fp32 adds).

On-core dataflow (all activations feature-major, i.e. transposed):
  XT [H, T] is loaded ONCE into SBUF (resident) and swept three times
  with weights streaming: K sweep, V sweep (token-major, i-outer so it
  pipelines against K's rope evictions), Q sweeps (one head per sweep,
  alternating PSUM rings).  RoPE is applied on PSUM eviction.

  Attention runs per 512-wide q-tile with a software pipeline over heads:
  at step h the PE emits S^T(h) tiles while the row-sum + AV matmuls of
  head h-1 consume the exp'd tiles, so the scalar engine's exp stream
  (the slow stage) is never on the PE's critical path.  Causal masking
  uses a single [128,128] additive diagonal block; fully-masked 128-col
  sub-blocks of band tiles are skipped via partial-width matmuls/exps.
  Softmax is max-free; 1/rowsum via reciprocal_approx_fast, broadcast
  through a PE outer product.

  The Wo projection is interleaved after q-tiles 1 and 3 (all heads'
  O^T columns for those tiles are complete), which keeps the PE warm and
  amortizes weight loads over two 512-wide rhs tiles.
"""

import math
import sys

import numpy as np

for _p in ("/opt/trn_rl_repo",):
    if _p not in sys.path:
        sys.path.insert(0, _p)

import ml_dtypes  # noqa: E402

import concourse.bass as bass  # noqa: E402
import concourse.mybir as mybir  # noqa: E402
import concourse.tile as tile  # noqa: E402
from concourse.alu_op_type import AluOpType  # noqa: E402

F32 = mybir.dt.float32
BF16 = mybir.dt.bfloat16
AF = mybir.ActivationFunctionType

# full problem constants
B, S, H = 2, 2048, 4096
NUM_HEADS, NUM_KV_HEADS, D = 32, 8, 128
ROPE_THETA = 10000.0
TP, DP = 4, 2
MASK_VAL = -30000.0


def build_nc(T=S, HID=H, NQL=NUM_HEADS // TP, NKVL=NUM_KV_HEADS // TP,
             HOUT=H, TQ=512):
    """One-core SPMD program.  T tokens, HID hidden, NQL local Q heads,
    NKVL local KV heads, HOUT output features, TQ q-tile width."""
    assert T % TQ == 0 and TQ % 128 == 0 and HID % 128 == 0
    GRP = NQL // NKVL            # q heads per kv head
    QC = NQL * D                 # local q columns
    KC = NKVL * D                # local kv columns
    KX = HID // 128              # contraction chunks for projections
    NTOK = T // TQ               # token tiles of width TQ
    NT128 = T // 128             # token tiles of width 128
    NKT = TQ // 128              # 128-wide k tiles per q tile
    inv_sqrt_d = 1.0 / math.sqrt(D)

    nc = bass.Bass()
    xt = nc.dram_tensor("xt", [HID, T], BF16, kind="ExternalInput")
    wq = nc.dram_tensor("wq", [HID, QC], BF16, kind="ExternalInput")
    wk = nc.dram_tensor("wk", [HID, KC], BF16, kind="ExternalInput")
    wv = nc.dram_tensor("wv", [HID, KC], BF16, kind="ExternalInput")
    wo = nc.dram_tensor("wo", [QC, HOUT], BF16, kind="ExternalInput")
    cosb = nc.dram_tensor("cosb", [128, T], BF16, kind="ExternalInput")
    sinb = nc.dram_tensor("sinb", [128, T], BF16, kind="ExternalInput")  # sign-folded
    diagb = nc.dram_tensor("diagb", [128, 128], F32, kind="ExternalInput")
    outp = nc.dram_tensor("outp", [HOUT, T], F32, kind="ExternalOutput")

    with tile.TileContext(nc) as tc:
        with (
            tc.tile_pool(name="resident", bufs=1) as res,
            tc.tile_pool(name="const", bufs=1) as const,
        ):
            # resident SBUF arrays (live across the whole kernel)
            qt = [res.tile([128, T], BF16, tag=f"qt{h}", name=f"qt{h}") for h in range(NQL)]
            kt = [res.tile([128, T], BF16, tag=f"kt{h}", name=f"kt{h}") for h in range(NKVL)]
            vt = [res.tile([128, KC], BF16, tag=f"v{t}", name=f"v{t}") for t in range(NT128)]
            cos_sb = res.tile([128, T], BF16, tag="cos")
            sin_sb = res.tile([128, T], BF16, tag="sin")
            diag_sb = res.tile([128, 128], F32, tag="diag")
            ones_col = const.tile([128, 1], BF16)
            ones_row = const.tile([1, 128], BF16)
            nc.vector.memset(ones_col[:], 1.0)
            nc.vector.memset(ones_row[:], 1.0)

            # ---------------- phase 1: projections ----------------
            with (
                tc.tile_pool(name="xres", bufs=1) as xres,
                tc.tile_pool(name="wt", bufs=1) as wpool,
                tc.tile_pool(name="rope_tmp", bufs=1) as rpool,
            ):
                # resident XT: one HBM read for all sweeps (DMAs emitted
                # inside the V sweep, interleaved with the wv chunks, so the
                # first matmuls aren't queued behind the whole 16MB load)
                xts = [xres.tile([128, T], BF16, tag=f"x{k}", name=f"x{k}")
                       for k in range(KX)]

                def rope_evict(ps, dst_ap, tok0):
                    """dst = ps*cos + rot_half(ps)*sin  (sin sign-folded)."""
                    cw = cos_sb[:, tok0:tok0 + TQ]
                    sw = sin_sb[:, tok0:tok0 + TQ]
                    r = rpool.tile([128, TQ], F32, tag="rot", bufs=2, name="rot")
                    nc.scalar.copy(r[0:64, :], ps[64:128, :])
                    nc.scalar.copy(r[64:128, :], ps[0:64, :])
                    t1 = rpool.tile([128, TQ], F32, tag="t1", bufs=2, name="t1")
                    nc.vector.tensor_tensor(t1[:], ps[:], cw, op=AluOpType.mult)
                    nc.vector.tensor_tensor(r[:], r[:], sw, op=AluOpType.mult)
                    nc.vector.tensor_tensor(dst_ap, t1[:], r[:], op=AluOpType.add)

                # V sweep first (no prior-eviction PSUM waits): token-major,
                # k-outer, two groups of 8 banks
                with tc.tile_pool(name="pv", bufs=1, space="PSUM") as pv:
                    for g in range(NT128 // 8):
                        psv = [pv.tile([128, KC], F32, tag=f"pv{i}", name=f"psv{i}")
                               for i in range(8)]
                        for k in range(KX):
                            if g == 0:
                                nc.sync.dma_start(out=xts[k][:],
                                                  in_=xt[k * 128:(k + 1) * 128, :])
                            wv_sb = wpool.tile([128, KC], BF16, tag="wv", bufs=6,
                                               name="wv_sb")
                            nc.sync.dma_start(out=wv_sb[:],
                                              in_=wv[k * 128:(k + 1) * 128, :])
                            for i in range(8):
                                t0 = (g * 8 + i) * 128
                                nc.tensor.matmul(
                                    psv[i][:], lhsT=xts[k][:, t0:t0 + 128],
                                    rhs=wv_sb[:],
                                    start=(k == 0), stop=(k == KX - 1))
                        if g == 0:
                            nc.sync.dma_start(out=cos_sb[:], in_=cosb[:])
                            nc.sync.dma_start(out=sin_sb[:], in_=sinb[:])
                            nc.sync.dma_start(out=diag_sb[:], in_=diagb[:])
                        for i in range(8):
                            nc.scalar.copy(vt[g * 8 + i][:], psv[i][:])

                # K sweep: one head per pass (4 banks), so pass i's rope
                # evictions drain behind pass i+1's matmuls
                with tc.tile_pool(name="pk", bufs=1, space="PSUM") as pk:
                    for i in range(NKVL):
                        psk = [pk.tile([128, TQ], F32, tag=f"pk{i % 2}{j}",
                                       name=f"pk{j}") for j in range(NTOK)]
                        for k in range(KX):
                            wk_sb = wpool.tile([128, KC], BF16, tag="wk", bufs=6,
                                               name="wk_sb")
                            nc.sync.dma_start(out=wk_sb[:],
                                              in_=wk[k * 128:(k + 1) * 128, :])
                            for j in range(NTOK):
                                nc.tensor.matmul(
                                    psk[j][:],
                                    lhsT=wk_sb[:, i * 128:(i + 1) * 128],
                                    rhs=xts[k][:, j * TQ:(j + 1) * TQ],
                                    start=(k == 0), stop=(k == KX - 1))
                        for j in range(NTOK):
                            rope_evict(psk[j], kt[i][:, j * TQ:(j + 1) * TQ], j * TQ)

                # Q sweeps: one head per sweep, 4 banks, alternating rings
                with tc.tile_pool(name="pq", bufs=1, space="PSUM") as pq:
                    for h in range(NQL):
                        psq = [pq.tile([128, TQ], F32, tag=f"pq{j}", bufs=2,
                                       name=f"psq{j}") for j in range(NTOK)]
                        for k in range(KX):
                            wq_sb = wpool.tile([128, 128], BF16, tag="wq", bufs=8,
                                               name="wq_sb")
                            nc.sync.dma_start(
                                out=wq_sb[:],
                                in_=wq[k * 128:(k + 1) * 128, h * 128:(h + 1) * 128])
                            for j in range(NTOK):
                                nc.tensor.matmul(
                                    psq[j][:], lhsT=wq_sb[:],
                                    rhs=xts[k][:, j * TQ:(j + 1) * TQ],
                                    start=(k == 0), stop=(k == KX - 1))
                        for j in range(NTOK):
                            rope_evict(psq[j], qt[h][:, j * TQ:(j + 1) * TQ], j * TQ)

            # -------- phase 2: attention, head-pair pipelined --------
            # Producer emits S^T for a pair of heads sharing each kt weight
            # load, as [128, 1024] ki-pair PSUM tiles (one exp per pair).
            # Consumer (previous pair) runs AV + row-sums, vt loads shared.
            # All 8 row-sum accumulators live in ONE PSUM bank as M=1
            # matmuls into distinct 32-partition column groups.
            CT = QC // 128  # Wo contraction chunks (== NQL)
            NPAIR = NQL // 2
            with (
                tc.tile_pool(name="ot_pool", bufs=1) as otpool,
                tc.tile_pool(name="es", bufs=1) as epool,
                tc.tile_pool(name="at_small", bufs=1) as spool,
                tc.tile_pool(name="ob", bufs=1) as obpool,
            ):
                ot = [otpool.tile([128, T], BF16, tag=f"ot{h}", name=f"ot{h}")
                      for h in range(NQL)]

                for qi in range(NTOK):
                    nk = (qi + 1) * NKT
                    npr = nk // 2  # ki pairs
                    q0 = qi * TQ
                    with (
                        tc.tile_pool(name="ps_s", bufs=1, space="PSUM") as psum_s,
                        tc.tile_pool(name="ps_o", bufs=1, space="PSUM") as psum_o,
                        tc.tile_pool(name="ps_n", bufs=1, space="PSUM") as psum_n,
                    ):
                        es = {}
                        pending = []

                        def col0(ki):
                            j = ki - qi * NKT
                            return 128 * j if j > 0 else 0

                        def flush_pending():
                            # 1/rowsum = exp(-ln(rowsum)) on ACT (the DVE
                            # reciprocal is an 8x-repeat iterative divide)
                            hp, po_, pn_ = pending.pop(0)
                            lr = spool.tile([1, TQ], F32, tag="lr", bufs=2,
                                            name="lr")
                            nc.scalar.activation(lr[:], pn_[0:1, :], AF.Ln)
                            rcb = spool.tile([1, TQ], BF16, tag="rcb", bufs=2,
                                             name="rcb")
                            nc.scalar.activation(rcb[:], lr[:], AF.Exp, scale=-1.0)
                            ps_b = psum_s.tile([128, 2 * TQ], F32, tag="sa",
                                               name="ps_b")
                            nc.tensor.matmul(ps_b[:, 0:TQ], lhsT=ones_row[:],
                                             rhs=rcb[:], start=True, stop=True)
                            bc = spool.tile([128, TQ], BF16, tag="bc", bufs=2,
                                            name="bc")
                            nc.vector.tensor_copy(bc[:], ps_b[:, 0:TQ])
                            nc.vector.tensor_tensor(
                                ot[hp][:, q0:q0 + TQ], po_[:], bc[:],
                                op=AluOpType.mult)

                        for rr in range(NPAIR + 2):
                            prod = rr if rr < NPAIR else None
                            cons = rr - 1 if 1 <= rr <= NPAIR else None
                            if prod is None:
                                # no S block to hide the flush chain behind
                                while pending:
                                    flush_pending()
                            po_a = po_b = pn_a = pn_b = None
                            if cons is not None:
                                ha, hb = 2 * cons, 2 * cons + 1
                                kvc = ha // GRP
                            if prod is not None:
                                pa_, pb_ = 2 * prod, 2 * prod + 1
                                kvp = pa_ // GRP
                            for p in range(npr):
                                if prod is not None:
                                    s2a = psum_s.tile([128, 2 * TQ], F32, tag="sa",
                                                      name="s2a")
                                    s2b = psum_s.tile([128, 2 * TQ], F32, tag="sb",
                                                      name="s2b")
                                    for sub in range(2):
                                        ki = 2 * p + sub
                                        c0 = col0(ki)
                                        for hh, s2 in ((pa_, s2a), (pb_, s2b)):
                                            nc.tensor.matmul(
                                                s2[:, sub * TQ + c0:(sub + 1) * TQ],
                                                lhsT=kt[kvp][:, ki * 128:(ki + 1) * 128],
                                                rhs=qt[hh][:, q0 + c0:q0 + TQ],
                                                start=True, stop=True)
                                        if ki >= qi * NKT:  # diagonal block
                                            for s2 in (s2a, s2b):
                                                nc.vector.tensor_tensor(
                                                    s2[:, sub * TQ + c0:
                                                       sub * TQ + c0 + 128],
                                                    s2[:, sub * TQ + c0:
                                                       sub * TQ + c0 + 128],
                                                    diag_sb[:], op=AluOpType.add)
                                    pc0 = col0(2 * p)
                                    for hh, s2 in ((pa_, s2a), (pb_, s2b)):
                                        e = epool.tile(
                                            [128, 2 * TQ], BF16,
                                            tag=f"e{prod % 2}{hh % 2}_{p}", name="e")
                                        nc.scalar.activation(
                                            e[:, pc0:2 * TQ], s2[:, pc0:2 * TQ],
                                            AF.Exp, scale=inv_sqrt_d)
                                        es[(hh, p)] = e
                                if p == 0:
                                    # flush behind the first S block so the
                                    # ACT Ln/Exp chain never heads the PE queue
                                    while pending:
                                        flush_pending()
                                    if cons is not None:
                                        po_a = psum_o.tile([128, TQ], F32,
                                                           tag="o", bufs=2,
                                                           name="po_a")
                                        po_b = psum_o.tile([128, TQ], F32,
                                                           tag="o", bufs=2,
                                                           name="po_b")
                                        pn_a = psum_n.tile([1, TQ], F32,
                                                           tag="n", bufs=2,
                                                           name="pn_a")
                                        pn_b = psum_n.tile([1, TQ], F32,
                                                           tag="n", bufs=2,
                                                           name="pn_b")
                                if cons is not None:
                                    for sub in range(2):
                                        ki = 2 * p + sub
                                        c0 = col0(ki)
                                        for hh, pn_ in ((ha, pn_a), (hb, pn_b)):
                                            nc.tensor.matmul(
                                                pn_[0:1, c0:TQ],
                                                lhsT=ones_col[:],
                                                rhs=es[(hh, p)][:, sub * TQ + c0:
                                                                (sub + 1) * TQ],
                                                start=(ki == 0),
                                                stop=(ki == nk - 1))
                                        for hh, po_ in ((ha, po_a), (hb, po_b)):
                                            nc.tensor.matmul(
                                                po_[:, c0:TQ],
                                                lhsT=vt[ki][:, kvc * D:(kvc + 1) * D],
                                                rhs=es[(hh, p)][:, sub * TQ + c0:
                                                                (sub + 1) * TQ],
                                                start=(ki == 0),
                                                stop=(ki == nk - 1))
                            if cons is not None:
                                pending.append((ha, po_a, pn_a))
                                pending.append((hb, po_b, pn_b))
                        while pending:
                            flush_pending()

                # -------- phase 3: Wo projection, streamed weight groups ----
                with (
                    tc.tile_pool(name="wog", bufs=1) as wogp,
                    tc.tile_pool(name="po", bufs=1, space="PSUM") as pop,
                ):
                    NG = 8
                    for g0 in range(0, HOUT // 128, NG):
                        wos_g = []
                        for c in range(CT):
                            w = wogp.tile([128, NG * 128], BF16, tag=f"wg{c}",
                                          bufs=2, name=f"wg{c}")
                            nc.sync.dma_start(
                                out=w[:],
                                in_=wo[c * 128:(c + 1) * 128,
                                       g0 * 128:(g0 + NG) * 128])
                            wos_g.append(w)
                        for i in range(NG):
                            ni = g0 + i
                            ps = [pop.tile([128, TQ], F32, tag=f"po{j}", bufs=2,
                                           name=f"po{j}") for j in range(NTOK)]
                            for c in range(CT):
                                for j in range(NTOK):
                                    nc.tensor.matmul(
                                        ps[j][:],
                                        lhsT=wos_g[c][:, i * 128:(i + 1) * 128],
                                        rhs=ot[c][:, j * TQ:(j + 1) * TQ],
                                        start=(c == 0), stop=(c == CT - 1))
                            for j in range(NTOK):
                                ob = obpool.tile([128, TQ], F32, tag="ob",
                                                 bufs=4, name="ob")
                                if j % 2 == 0:
                                    nc.scalar.copy(ob[:], ps[j][:])
                                else:
                                    nc.vector.tensor_copy(ob[:], ps[j][:])
                                nc.sync.dma_start(
                                    out=outp[ni * 128:(ni + 1) * 128,
                                             j * TQ:(j + 1) * TQ],
                                    in_=ob[:])
    legalize_wait_counts(nc)
    return nc


def legalize_wait_counts(nc):
    """walrus DIRECT2D descriptors accept a single sync-wait; Tile can emit
    more (data wait + queue-head wait).  Hoist excess waits onto
    EventSemaphore instructions inserted just before, on the same engine."""
    n_new = 0
    for f in nc.m.functions:
        for blk in f.blocks:
            idx = 0
            insts = blk.instructions
            while idx < len(insts):
                inst = insts[idx]
                si = getattr(inst, "sync_info", None)
                cap = 2 if isinstance(inst, mybir.InstEventSemaphore) else 1
                waits = list(si.on_wait) if si is not None and si.on_wait else []
                if len(waits) > cap:
                    keep, extra = waits[-cap:], waits[:-cap]
                    si.on_wait = keep
                    for i in range(0, len(extra), 2):
                        ev = mybir.InstEventSemaphore(
                            name=f"waitsplit_{n_new}", ins=[], outs=[])
                        n_new += 1
                        ev.engine = inst.engine
                        ev.sync_info = mybir.SyncInfo(
                            on_wait=extra[i:i + 2], on_update=[])
                        nc.register_instruction(ev)
                        insts.insert(idx, ev)
                        idx += 1
                idx += 1
    return n_new


def _host_inputs(hidden_states, position_ids, Wq, Wk, Wv, Wo):
    """Build the 8 per-core input maps."""
    hs = np.asarray(hidden_states, dtype=np.float32)
    pos = np.asarray(position_ids)
    Wq = np.asarray(Wq, dtype=np.float32)
    Wk = np.asarray(Wk, dtype=np.float32)
    Wv = np.asarray(Wv, dtype=np.float32)
    Wo = np.asarray(Wo, dtype=np.float32)
    b, s, h = hs.shape
    qc = h // TP
    kc = (NUM_KV_HEADS * D) // TP
    bf = ml_dtypes.bfloat16

    # rope tables per batch, feature-major, sin sign-folded for rotate_half
    inv_freq = 1.0 / (ROPE_THETA ** (np.arange(0, D, 2, dtype=np.float32) / D))
    maps = []
    i_idx = np.arange(128)[:, None]
    c_idx = np.arange(128)[None, :]
    diagb = np.where(c_idx >= i_idx, 0.0, MASK_VAL).astype(np.float32)

    for c in range(DP * TP):
        bb, r = c // TP, c % TP
        t = pos[bb].astype(np.float64)  # [s]
        ang = t[None, :] * np.concatenate([inv_freq, inv_freq])[:, None]  # [128, s]
        cosb = np.cos(ang).astype(np.float32)
        sinb = np.sin(ang).astype(np.float32)
        sinb[0:64, :] *= -1.0  # rotate_half sign fold
        maps.append({
            "xt": np.ascontiguousarray(hs[bb].T).astype(bf),
            "wq": np.ascontiguousarray(Wq[:, r * qc:(r + 1) * qc]).astype(bf),
            "wk": np.ascontiguousarray(Wk[:, r * kc:(r + 1) * kc]).astype(bf),
            "wv": np.ascontiguousarray(Wv[:, r * kc:(r + 1) * kc]).astype(bf),
            "wo": np.ascontiguousarray(Wo[r * qc:(r + 1) * qc, :]).astype(bf),
            "cosb": cosb.astype(bf),
            "sinb": sinb.astype(bf),
            "diagb": diagb,
        })
    return maps


_NC_CACHE = {}


def _get_nc():
    if "nc" not in _NC_CACHE:
        _NC_CACHE["nc"] = build_nc()
    return _NC_CACHE["nc"]


def kernel(hidden_states, position_ids, Wq, Wk, Wv, Wo, _results_hook=None):
    from concourse.bass_utils import run_bass_kernel_spmd

    maps = _host_inputs(hidden_states, position_ids, Wq, Wk, Wv, Wo)
    nc = _get_nc()
    res = run_bass_kernel_spmd(nc, maps, list(range(DP * TP)))
    if _results_hook is not None:
        _results_hook(res)
    b, s, h = np.asarray(hidden_states).shape
    out = np.zeros((b, s, h), dtype=np.float32)
    for c in range(DP * TP):
        bb = c // TP
        out[bb] += res.results[c]["outp"].T
    return out


if __name__ == "__main__":
    # smoke: build the full-size program and print instruction counts
    nc = build_nc()
    print("built ok")
